# revision 1
# baseline (speedup 1.0000x reference)
"""Trainium2 Bass kernel for nn_Architecture_50629074485965 (3-layer AKT-style
transformer, B=16 S=512 D=1024 H=8 DFF=4096).

Sharding: data-parallel over batch — 2 batches per core, 8 cores, no
collectives.  Activations are feature-major [D on partitions, tokens free] so
every matmul chains without activation transposes (weights host-pre-
transposed).  Matmuls run in float32r (TF32-like, ~1.6e-4 rel err, 4x fp32
rate).  FFN hidden + w2 in bf16.  Layer outputs bounce through DRAM.

The problem spec pins all biases to zeros and LN affines to identity, so those
terms are skipped.

Attention per (b,h), per 128-row q-tile (q-major [q, k] layout):
  psum  = q @ k^T                         (PE f32r)
  e1    = Exp(psum/sqrt(dk))              (ACT, full width)
  e1c   = causal(e1)                      (GPSIMD affine_select, width w)
  r1    = sum_j e1*dam01                  (DVE stt accum, e1 in-place;
                                           dam01 = u8 [128,512] row-window
                                           gather from a per-head Toeplitz
                                           vector via indirect_dma_start)
  cum   = cumsum(e1c)                     (DVE tensor_tensor_scan)
  d2    = (cum - rowtot) * (-|i-j|) >= 0  (DVE stt, posn = -|i-j| in f16)
  dist  = Sqrt(d2 * (1/r1))               (ACT, scale AP)
  te    = Exp(dist * -softplus(gamma))    (ACT, scale AP)
  t2u   = max(te,1e-5) * psum             (DVE stt)
  t2m   = causal(t2u, fill=-1e30)         (GPSIMD affine_select)
  e2,r2 = Exp(t2m/sqrt(dk)) + row-sum     (ACT accum_out)
  probs = e2 * (1/max(r2,1e-30)) -> f32r  (DVE)
  probsT blocks: PE transpose -> psum -> sbuf (ACT copies)
  att   = v-chunks(lhsT) @ probsT -> feature-major  (PE)
"""
import sys
sys.path.insert(0, "/opt/trn_rl_repo")
import numpy as np

B, S, D, H, DFF, LN_ = 16, 512, 1024, 8, 4096, 3
DK = D // H
NB = 2
TOK = NB * S
P = 128
ND = D // P      # 8
NQ = S // P      # 4
ISD = 1.0 / float(np.sqrt(DK))
WPAD = 2048

_CACHE = {}


def _build(nlayers=3, taps=(), repeat=1):
    import concourse.bass as bass
    import concourse.mybir as mybir
    from concourse import bacc
    from concourse.tile import TileContext

    dt = mybir.dt
    f32, f32r, bf16, f16, u8, i32 = (dt.float32, dt.float32r, dt.bfloat16,
                                     dt.float16, dt.uint8, dt.int32)
    AF = mybir.ActivationFunctionType
    OP = mybir.AluOpType

    nc = bacc.Bacc(None, target_bir_lowering=False)

    def par(name, shape, out=False, dtype=None):
        return nc.declare_dram_parameter(name, list(shape), dtype or f32,
                                         isOutput=out)

    xqa_e = par("xqa", [D, TOK], dtype=f32r)
    xq_e = par("xq", [D, TOK], dtype=f32r)
    kwt_e = par("kwt", [LN_, D, D], dtype=f32r)
    vwt_e = par("vwt", [LN_, D, D], dtype=f32r)
    owt_e = par("owt", [LN_, D, D], dtype=f32r)
    w1t_e = par("w1t", [LN_, D, DFF], dtype=bf16)
    w2t_e = par("w2t", [LN_, DFF, D], dtype=bf16)
    a0f_e = par("a0f", [LN_, H, S]); a1f_e = par("a1f", [LN_, H, S])
    e0f_e = par("e0f", [LN_, H, S]); e1f_e = par("e1f", [LN_, H, S])
    a0r_e = par("a0r", [LN_, H, S]); a1r_e = par("a1r", [LN_, H, S])
    e0r_e = par("e0r", [LN_, H, S]); e1r_e = par("e1r", [LN_, H, S])
    gam_e = par("gam", [1, LN_ * H])
    posn_e = par("posn", [S, S], dtype=f16)
    out_e = par("out", [D, TOK], out=True)
    tap_outs = {}

    with TileContext(nc) as tc:
        pg = tc.alloc_tile_pool(name="glob", bufs=1)
        pdram = tc.alloc_tile_pool(name="dram", bufs=1, space="DRAM")
        psQ = tc.alloc_tile_pool(name="psQ", bufs=2, space="PSUM")
        psT = tc.alloc_tile_pool(name="psT", bufs=1, space="PSUM")
        psAv = tc.alloc_tile_pool(name="psAv", bufs=1, space="PSUM")

        _dmaq = [nc.sync, nc.scalar]
        _dmac = [0]

        def wdma(out, in_):
            eng = _dmaq[_dmac[0] % len(_dmaq)]
            _dmac[0] += 1
            eng.dma_start(out=out, in_=in_)

        def mm_group(psum_ap, pairs):
            n = len(pairs)
            for i, (lt, rh) in enumerate(pairs):
                nc.tensor.matmul(psum_ap, lt, rh,
                                 start=(i == 0), stop=(i == n - 1))

        # ---------------- constants (global pool) ----------------
        ident_f = pg.tile([P, P], f32, name="t", tag="identf")
        nc.gpsimd.memset(ident_f[:], 0.0)
        nc.gpsimd.affine_select(
            out=ident_f[:], in_=ident_f[:], compare_op=OP.not_equal,
            fill=1.0, base=0, channel_multiplier=1, pattern=[[-1, P]])
        ident = pg.tile([P, P], f32r, name="t", tag="ident")
        nc.vector.tensor_copy(ident[:], ident_f[:])

        ones_f = pg.tile([P, 1], f32, name="t", tag="onesf")
        nc.gpsimd.memset(ones_f[:], 1.0)
        ones_col = pg.tile([P, 1], f32r, name="t", tag="ones")
        nc.vector.tensor_copy(ones_col[:], ones_f[:])
        eps5 = pg.tile([P, 1], f32, name="t", tag="eps5")
        nc.gpsimd.memset(eps5[:], 1e-5)

        posn = []
        for qt in range(NQ):
            t = pg.tile([P, S], f16, name="t", tag=f"posn{qt}")
            nc.sync.dma_start(out=t[:], in_=posn_e[qt * P:(qt + 1) * P, :])
            posn.append(t)

        idxt = []
        for h in range(H):
            t = pg.tile([P, 1], i32, name="t", tag=f"idx{h}")
            nc.gpsimd.iota(t[:], pattern=[[1, 1]],
                           base=h * WPAD + (S - 1) - P * (NQ - 1),
                           channel_multiplier=-1)
            idxt.append(t)

        grow = pg.tile([1, LN_ * H], f32, name="t", tag="grow")
        nc.sync.dma_start(out=grow[:], in_=gam_e[:])
        one_c = pg.tile([P, 1], f32, name="t", tag="one_c")
        nc.gpsimd.memset(one_c[:], 1.0)
        # softplus(x) = ln(1 + exp(x)) computed manually (no Softplus table)
        gsp = pg.tile([1, LN_ * H], f32, name="t", tag="gsp")
        nc.scalar.activation(gsp[:], grow[:], AF.Exp)
        nc.scalar.activation(gsp[:], gsp[:], AF.Ln, bias=one_c[:1, :])
        gneg = pg.tile([1, LN_ * H], f32, name="t", tag="gneg")
        nc.vector.tensor_scalar(gneg[:], gsp[:], -1.0, None, OP.mult)
        gam_bc = []
        for i in range(LN_ * H):
            t = pg.tile([P, 1], f32, name="t", tag=f"gbc{i}")
            nc.gpsimd.partition_broadcast(t[:], gneg[0:1, i:i + 1])
            gam_bc.append(t)

        y_dram = pdram.tile([D, TOK], f32r, name="t", tag="ydram")
        x1_dram = pdram.tile([D, TOK], f32r, name="t", tag="x1dram")

        # ---------------- helpers ----------------
        def dam_prep(l):
            wdam = pdram.tile([1, H * WPAD], u8, name="t", tag="wdam")
            pp = tc.alloc_tile_pool(name=f"dp{l}", bufs=1)

            def half(a0e, a1e, e0e, e1e):
                tA = pp.tile([H, S], f32, name="t", tag="dpA")
                tB = pp.tile([H, S], f32, name="t", tag="dpB")
                tC = pp.tile([H, S], f32, name="t", tag="dpC")
                tD = pp.tile([H, S], f32, name="t", tag="dpD")
                nc.sync.dma_start(out=tA[:], in_=e0e[l])
                nc.sync.dma_start(out=tB[:], in_=e1e[l])
                nc.scalar.activation(tA[:], tA[:], AF.Ln, bias=eps5[:H, :])
                nc.scalar.activation(tB[:], tB[:], AF.Ln, bias=eps5[:H, :])
                nc.vector.tensor_tensor(tA[:], tA[:], tB[:], OP.subtract)
                nc.sync.dma_start(out=tC[:], in_=a1e[l])
                nc.sync.dma_start(out=tD[:], in_=a0e[l])
                nc.vector.tensor_tensor(tC[:], tC[:], tD[:], OP.subtract)
                nc.vector.tensor_tensor(tA[:], tA[:], tC[:], OP.add)
                c = pp.tile([H, S], u8, name="t", tag="dpc", bufs=2)
                nc.vector.tensor_scalar(c[:], tA[:], 0.0, None, OP.is_gt)
                return c

            cf = half(a0f_e, a1f_e, e0f_e, e1f_e)
            cr = half(a0r_e, a1r_e, e0r_e, e1r_e)
            dst_r = bass.AP(tensor=wdam.tensor, offset=0,
                            ap=[[WPAD, H], [1, S - 1]])
            dst_f = bass.AP(tensor=wdam.tensor, offset=S - 1,
                            ap=[[WPAD, H], [1, S]])
            nc.sync.dma_start(out=dst_r, in_=cr[:, 0:S - 1])
            nc.sync.dma_start(out=dst_f, in_=cf[:])
            pp.release()
            return wdam

        def layernorm(pool, r_t, dsts):
            """r_t: 8 [P,S] f32r tiles; writes (x-mu)/sigma into dsts APs."""
            s1 = psT.tile([1, S], f32, name="t", tag="pt0")
            mm_group(s1[:], [(ones_col[:], r_t[od][:]) for od in range(ND)])
            s2 = psT.tile([1, S], f32, name="t", tag="pt1")
            for od in range(ND):
                sq = pool.tile([P, S], f32r, name="t", tag="sqtmp", bufs=1)
                nc.vector.tensor_tensor(sq[:], r_t[od][:], r_t[od][:],
                                        OP.mult)
                nc.tensor.matmul(s2[:], ones_col[:], sq[:],
                                 start=(od == 0), stop=(od == ND - 1))
            mean = pool.tile([1, S], f32, name="t", tag="lnr0", bufs=1)
            nc.vector.tensor_scalar(mean[:], s1[:], 1.0 / D, None, OP.mult)
            msq = pool.tile([1, S], f32, name="t", tag="lnr1", bufs=1)
            nc.vector.tensor_scalar(msq[:], s2[:], 1.0 / D, None, OP.mult)
            m2 = pool.tile([1, S], f32, name="t", tag="lnr2", bufs=1)
            nc.vector.tensor_tensor(m2[:], mean[:], mean[:], OP.mult)
            nc.vector.tensor_tensor(msq[:], msq[:], m2[:], OP.subtract)
            nc.scalar.activation(msq[:], msq[:], AF.Sqrt, bias=eps5[:1, :])
            nc.vector.reciprocal(m2[:], msq[:])          # m2 = rstd
            nc.vector.tensor_scalar(mean[:], mean[:], -1.0, None, OP.mult)
            nc.vector.tensor_tensor(mean[:], mean[:], m2[:], OP.mult)
            Ab = pool.tile([P, S], f32, name="t", tag="Ab", bufs=1)
            nc.gpsimd.partition_broadcast(Ab[:], m2[:])
            Cb = pool.tile([P, S], f32, name="t", tag="Cb", bufs=1)
            nc.gpsimd.partition_broadcast(Cb[:], mean[:])
            for od in range(ND):
                t1 = pool.tile([P, S], f32, name="t", tag="lnt", bufs=1)
                nc.vector.tensor_tensor(t1[:], r_t[od][:], Ab[:], OP.mult)
                nc.gpsimd.tensor_tensor(dsts[od], t1[:], Cb[:], OP.add)

        def attention_head(pool, l, bmask, h, K, V, att_dst, damG):
            pst = [psT.tile([P, S], f32r, name="t", tag=f"pt{kc}")
                   for kc in range(NQ)]
            ktile = K[h]
            for qt in range(NQ):
                w = P * (qt + 1)
                ps = psQ.tile([P, S], f32, name="t", tag="qk")
                mm_group(ps[:], [(ktile[:, qt * P:qt * P + P], ktile[:])])
                doff = P * (NQ - 1) - P * qt
                e1 = pool.tile([P, S], f32, name="t", tag="e1")
                nc.scalar.activation(e1[:], ps[:], AF.Exp, scale=ISD)
                e1c = pool.tile([P, S], f32, name="t", tag="tmpA", bufs=4)
                nc.gpsimd.affine_select(
                    out=e1c[:, :w], in_=e1[:, :w], compare_op=OP.is_gt,
                    fill=0.0, base=qt * P + bmask, channel_multiplier=1,
                    pattern=[[-1, w]])
                r1 = pool.tile([P, 1], f32, name="t", tag="sm_r1")
                nc.vector.scalar_tensor_tensor(
                    e1[:], e1[:], 1.0, damG[:, doff:doff + S],
                    OP.mult, OP.mult, accum_out=r1[:])
                cum = pool.tile([P, S], f32, name="t", tag="tmpB", bufs=3)
                nc.vector.tensor_tensor_scan(
                    cum[:, :w], e1c[:, :w], e1c[:, :w], 0.0, OP.add, OP.bypass)
                rec1 = pool.tile([P, 1], f32, name="t", tag="sm_rc1")
                nc.vector.reciprocal(rec1[:], r1[:])
                d2 = pool.tile([P, S], f32, name="t", tag="tmpA", bufs=4)
                nc.vector.scalar_tensor_tensor(
                    d2[:, :w], cum[:, :w], cum[:, w - 1:w], posn[qt][:, :w],
                    OP.subtract, OP.mult)
                dist = pool.tile([P, S], f32, name="t", tag="tmpB", bufs=3)
                nc.scalar.activation(dist[:, :w], d2[:, :w], AF.Sqrt,
                                     scale=rec1[:])
                te = pool.tile([P, S], f32, name="t", tag="tmpA", bufs=4)
                nc.scalar.activation(te[:, :w], dist[:, :w], AF.Exp,
                                     scale=gam_bc[l * H + h][:])
                t2u = pool.tile([P, S], f32, name="t", tag="tmpB", bufs=3)
                nc.vector.scalar_tensor_tensor(
                    t2u[:, :w], te[:, :w], 1e-5, ps[:, :w], OP.max, OP.mult)
                t2m = pool.tile([P, S], f32, name="t", tag="tmpA", bufs=4)
                nc.gpsimd.affine_select(
                    out=t2m[:, :w], in_=t2u[:, :w], compare_op=OP.is_gt,
                    fill=-1e30, base=qt * P + bmask, channel_multiplier=1,
                    pattern=[[-1, w]])
                e2 = pool.tile([P, S], f32, name="t", tag="tmpB", bufs=3)
                r2 = pool.tile([P, 1], f32, name="t", tag="sm_r2")
                nc.scalar.activation(e2[:, :w], t2m[:, :w], AF.Exp,
                                     scale=ISD, accum_out=r2[:])
                nc.vector.tensor_scalar(r2[:], r2[:], 1e-30, None, OP.max)
                rec2 = pool.tile([P, 1], f32, name="t", tag="sm_rc2")
                nc.vector.reciprocal(rec2[:], r2[:])
                pr = pool.tile([P, S], f32r, name="t", tag="probs", bufs=2)
                nc.vector.tensor_scalar(pr[:, :w], e2[:, :w], rec2[:],
                                        None, OP.mult)
                for kc in range(qt + 1):
                    nc.tensor.transpose(
                        pst[kc][:, qt * P:qt * P + P],
                        pr[:, kc * P:kc * P + P], ident[:])
            prT = []
            for kc in range(NQ):
                t = pool.tile([P, S], f32r, name="t", tag=f"prT{kc}", bufs=1)
                nc.vector.tensor_copy(t[:, kc * P:], pst[kc][:, kc * P:])
                prT.append(t)
            pav = psAv.tile([P, S], f32, name="t", tag="av")
            for kc in range(NQ):
                nc.tensor.matmul(
                    pav[:, kc * P:], V[kc][:, h * DK:(h + 1) * DK],
                    prT[kc][:, kc * P:],
                    start=(kc == 0), stop=(kc == NQ - 1))
            nc.vector.tensor_copy(att_dst, pav[:])

        def layer(l, bmask, apply_pos, xq_src, vals_src, out_dram,
                  final=False):
            """xq_src: 8 [P,TOK] f32r tiles (query/key input).
            vals_src: 'self' or a DRAM tile to stream per b.
            out_dram: DRAM target AP base for the layer output."""
            wdam = dam_prep(l)
            pdam = tc.alloc_tile_pool(name=f"dam{l}", bufs=1)
            damGs = []
            for h in range(H):
                g = pdam.tile([P, 2 * S - 1], u8, name="t", tag=f"damG{h}")
                nc.gpsimd.indirect_dma_start(
                    out=g[:], out_offset=None, in_=wdam[:],
                    in_offset=bass.IndirectOffsetOnAxis(
                        ap=idxt[h][:, :1], axis=1))
                damGs.append(g)
            for b in range(NB):
                bs = b * S
                pool = tc.alloc_tile_pool(name=f"att{l}{b}", bufs=2)
                # ---- K projection (q==k), kwt streamed in od-halves
                K = []
                for half in range(2):
                    wk = []
                    for idt in range(ND):
                        t = pool.tile([P, S], f32r, name="t", tag=f"wbig{idt}",
                                          bufs=2)
                        wdma(
                            t[:],
                            kwt_e[l, idt * P:(idt + 1) * P,
                                      half * S:(half + 1) * S])
                        wk.append(t)
                    for oc in range(4):
                        od = half * 4 + oc
                        ps = psQ.tile([P, S], f32, name="t", tag="qk")
                        mm_group(ps[:], [
                            (wk[idt][:, oc * P:(oc + 1) * P],
                             xq_src[idt][:, bs:bs + S]) for idt in range(ND)])
                        kt = pool.tile([P, S], f32r, name="t", tag=f"K{od}",
                                       bufs=1)
                        nc.vector.tensor_copy(kt[:], ps[:])
                        K.append(kt)
                # ---- VALS for v-projection
                if vals_src == "self":
                    vals = [xq_src[idt][:, bs:bs + S] for idt in range(ND)]
                else:
                    vt = []
                    for idt in range(ND):
                        t = pool.tile([P, S], f32r, name="t", tag=f"att{idt}", bufs=1)
                        wdma(
                            t[:],
                            vals_src[idt * P:(idt + 1) * P, bs:bs + S])
                        vt.append(t)
                    vals = [t[:] for t in vt]
                # ---- V projection (token-major), vwt streamed in d-halves
                V = [pool.tile([P, D], f32r, name="t", tag=f"V{st}", bufs=1)
                     for st in range(NQ)]
                for half in range(2):
                    wv = []
                    for idt in range(ND):
                        t = pool.tile([P, S], f32r, name="t", tag=f"wbig{idt}",
                                          bufs=2)
                        wdma(
                            t[:],
                            vwt_e[l, idt * P:(idt + 1) * P,
                                      half * S:(half + 1) * S])
                        wv.append(t)
                    for st in range(NQ):
                        ps = psQ.tile([P, S], f32, name="t", tag="qk")
                        mm_group(ps[:], [
                            (vals[idt][:, st * P:(st + 1) * P], wv[idt][:])
                            for idt in range(ND)])
                        nc.vector.tensor_copy(
                            V[st][:, half * S:(half + 1) * S], ps[:])
                # ---- attention heads
                att = [pool.tile([P, S], f32r, name="t", tag=f"att{od}", bufs=1)
                       for od in range(ND)]
                for h in range(H):
                    attention_head(pool, l, bmask, h, K, V, att[h][:], damGs[h])
                # ---- o-projection + residual, owt streamed in od-halves
                r_t = []
                for half in range(2):
                    wo = []
                    for idt in range(ND):
                        t = pool.tile([P, S], f32r, name="t", tag=f"wbig{idt}",
                                          bufs=2)
                        wdma(
                            t[:],
                            owt_e[l, idt * P:(idt + 1) * P,
                                      half * S:(half + 1) * S])
                        wo.append(t)
                    for oc in range(4):
                        od = half * 4 + oc
                        ps = psQ.tile([P, S], f32, name="t", tag="qk")
                        mm_group(ps[:], [
                            (wo[idt][:, oc * P:(oc + 1) * P], att[idt][:])
                            for idt in range(ND)])
                        rt = pool.tile([P, S], f32r, name="t", tag=f"r{od}",
                                       bufs=1)
                        nc.vector.tensor_tensor(
                            rt[:], xq_src[od][:, bs:bs + S], ps[:], OP.add)
                        r_t.append(rt)
                # ---- LN1
                if apply_pos:
                    xp = [pg.tile([P, S], f32r, name="t", tag=f"xp{od}")
                          for od in range(ND)]
                    layernorm(pool, r_t, [t[:] for t in xp])
                else:
                    ot = [pool.tile([P, S], f32 if final else f32r, name="t",
                                    tag="outt", bufs=2)
                          for _ in range(ND)]
                    layernorm(pool, r_t, [t[:] for t in ot])
                    for od in range(ND):
                        nc.sync.dma_start(
                            out=out_dram[od * P:(od + 1) * P, bs:bs + S],
                            in_=ot[od][:])
                pool.release()

                if not apply_pos:
                    continue
                # ---- FFN + LN2
                fp = tc.alloc_tile_pool(name=f"ffn{l}{b}", bufs=2)
                xpb = []
                for od in range(ND):
                    t = fp.tile([P, S], bf16, name="t", tag=f"xpb{od}", bufs=1)
                    nc.vector.tensor_copy(t[:], xp[od][:])
                    xpb.append(t)
                h1 = []
                for fc in range(8):
                    w1c = []
                    for idt in range(ND):
                        t = fp.tile([P, S], bf16, name="t", tag=f"w1c{idt}")
                        wdma(
                            t[:],
                            w1t_e[l, idt * P:(idt + 1) * P,
                                      fc * S:(fc + 1) * S])
                        w1c.append(t)
                    for fl in range(4):
                        ps = psQ.tile([P, S], f32, name="t", tag="qk")
                        mm_group(ps[:], [
                            (w1c[idt][:, fl * P:(fl + 1) * P], xpb[idt][:])
                            for idt in range(ND)])
                        ht = fp.tile([P, S], bf16, name="t",
                                     tag=f"h1_{fc * 4 + fl}", bufs=1)
                        nc.vector.tensor_scalar(ht[:], ps[:], 0.0, None,
                                                OP.max)
                        h1.append(ht)
                r_t = []
                for og in range(2):
                    pso = [psT.tile([P, S], f32, name="t", tag=f"pt{oc}")
                           for oc in range(4)]
                    for fc in range(8):
                        w2c = []
                        for fl in range(4):
                            ft = fc * 4 + fl
                            t = fp.tile([P, S], bf16, name="t", tag=f"w2c{fl}")
                            wdma(
                                t[:],
                                w2t_e[l, ft * P:(ft + 1) * P,
                                          og * S:(og + 1) * S])
                            w2c.append(t)
                        for fl in range(4):
                            ft = fc * 4 + fl
                            for oc in range(4):
                                nc.tensor.matmul(
                                    pso[oc][:],
                                    w2c[fl][:, oc * P:(oc + 1) * P],
                                    h1[ft][:],
                                    start=(fc == 0 and fl == 0),
                                    stop=(fc == 7 and fl == 3))
                    for oc in range(4):
                        od = og * 4 + oc
                        rt = fp.tile([P, S], f32r, name="t", tag=f"r{od}",
                                     bufs=1)
                        nc.vector.tensor_tensor(
                            rt[:], xp[od][:], pso[oc][:], OP.add)
                        r_t.append(rt)
                ot = [fp.tile([P, S], f32 if final else f32r, name="t",
                              tag="outt", bufs=4)
                      for _ in range(ND)]
                layernorm(fp, r_t, [t[:] for t in ot])
                for od in range(ND):
                    nc.sync.dma_start(
                        out=out_dram[od * P:(od + 1) * P, bs:bs + S],
                        in_=ot[od][:])
                fp.release()
            pdam.release()

        def load_x(src):
            tiles = []
            for od in range(ND):
                t = pg.tile([P, TOK], f32r, name="t", tag=f"xa{od}")
                nc.sync.dma_start(out=t[:], in_=src[od * P:(od + 1) * P, :])
                tiles.append(t)
            return tiles

        # ================= driver =================
        for _rep in range(repeat):
            XA = load_x(xqa_e)
            layer(0, 1, True, XA, "self", y_dram)
            if nlayers >= 2:
                XA = load_x(xq_e)
                layer(1, 1, False, XA, "self", x1_dram)
            if nlayers >= 3:
                XA = load_x(x1_dram)
                layer(2, 0, True, XA, y_dram, out_e, final=True)
            if nlayers == 1:
                nc.gpsimd.dma_start(out=out_e[:], in_=y_dram[:])
            elif nlayers == 2:
                nc.gpsimd.dma_start(out=out_e[:], in_=x1_dram[:])

        psAv.release()
        psT.release()
        psQ.release()
        pdram.release()
        pg.release()

    nc.finalize()
    return nc, tap_outs


def _get_nc(nlayers=3, taps=(), repeat=1):
    key = (nlayers, tuple(sorted(taps)), repeat)
    if key not in _CACHE:
        _CACHE[key] = _build(nlayers, taps, repeat)
    return _CACHE[key]


def _make_in_maps(inputs):
    qa = np.asarray(inputs["qa_embed_data"])
    qd = np.asarray(inputs["q_embed_data"])
    al = np.asarray(inputs["alphas"])
    ge = np.asarray(inputs["gumbel_E"])
    a0f = al[..., 0]; a1f = al[..., 1]
    e0f = ge[..., 0]; e1f = ge[..., 1]
    i_ = np.arange(S)
    shared = {
        "kwt": np.asarray(inputs["kW"]).transpose(0, 2, 1),
        "vwt": np.asarray(inputs["vW"]).transpose(0, 2, 1),
        "owt": np.asarray(inputs["oW"]).transpose(0, 2, 1),
        "w1t": np.asarray(inputs["w1"]).transpose(0, 2, 1),
        "w2t": np.asarray(inputs["w2"]).transpose(0, 2, 1),
        "a0f": a0f, "a1f": a1f, "e0f": e0f, "e1f": e1f,
        "a0r": a0f[:, :, ::-1], "a1r": a1f[:, :, ::-1],
        "e0r": e0f[:, :, ::-1], "e1r": e1f[:, :, ::-1],
        "gam": np.asarray(inputs["gammas"]).reshape(1, LN_ * H),
        "posn": -np.abs(i_[:, None] - i_[None, :]),
    }
    import ml_dtypes
    casts = {"w1t": ml_dtypes.bfloat16, "w2t": ml_dtypes.bfloat16,
             "posn": np.float16}
    shared = {k: np.ascontiguousarray(v, dtype=casts.get(k, np.float32))
              for k, v in shared.items()}

    def feat_major(x, c):
        pair = np.asarray(x[NB * c:NB * c + NB])        # [2, S, D]
        return np.ascontiguousarray(
            pair.transpose(2, 0, 1).reshape(D, TOK), dtype=np.float32)

    in_maps = []
    for c in range(8):
        m = dict(shared)
        m["xqa"] = feat_major(qa, c)
        m["xq"] = feat_major(qd, c)
        in_maps.append(m)
    return in_maps


def _gather_out(results):
    outs = []
    for r in results:
        o = r["out"].reshape(D, NB, S).transpose(1, 2, 0)
        outs.append(o)
    return np.ascontiguousarray(np.concatenate(outs, axis=0))


def kernel(**inputs):
    from concourse.bass_utils import run_bass_kernel_spmd
    nc, _ = _get_nc()
    in_maps = _make_in_maps(inputs)
    res = run_bass_kernel_spmd(nc, in_maps, core_ids=list(range(8)))
    return _gather_out(res.results)



# revision 6
# speedup vs baseline: 1.0549x; 1.0549x over previous
"""Trainium2 Bass kernel for nn_Architecture_50629074485965 (3-layer AKT-style
transformer, B=16 S=512 D=1024 H=8 DFF=4096).

Sharding: data-parallel over batch — 2 batches per core, 8 cores, no
collectives.  Activations are feature-major [D on partitions, tokens free] so
every matmul chains without activation transposes (weights host-pre-
transposed).  Matmuls run in float32r (TF32-like, ~1.6e-4 rel err, 4x fp32
rate).  FFN hidden + w2 in bf16.  Layer outputs bounce through DRAM.

The problem spec pins all biases to zeros and LN affines to identity, so those
terms are skipped.

Attention per (b,h), per 128-row q-tile (q-major [q, k] layout):
  psum  = q @ k^T                         (PE f32r)
  e1    = Exp(psum/sqrt(dk))              (ACT, full width)
  e1c   = causal(e1)                      (GPSIMD affine_select, width w)
  r1    = sum_j e1*dam01                  (DVE stt accum, e1 in-place;
                                           dam01 = u8 [128,512] row-window
                                           gather from a per-head Toeplitz
                                           vector via indirect_dma_start)
  cum   = cumsum(e1c)                     (DVE tensor_tensor_scan)
  d2    = (cum - rowtot) * (-|i-j|) >= 0  (DVE stt, posn = -|i-j| in f16)
  dist  = Sqrt(d2 * (1/r1))               (ACT, scale AP)
  te    = Exp(dist * -softplus(gamma))    (ACT, scale AP)
  t2u   = max(te,1e-5) * psum             (DVE stt)
  t2m   = causal(t2u, fill=-1e30)         (GPSIMD affine_select)
  e2,r2 = Exp(t2m/sqrt(dk)) + row-sum     (ACT accum_out)
  probs = e2 * (1/max(r2,1e-30)) -> f32r  (DVE)
  probsT blocks: PE transpose -> psum -> sbuf (ACT copies)
  att   = v-chunks(lhsT) @ probsT -> feature-major  (PE)
"""
import sys
sys.path.insert(0, "/opt/trn_rl_repo")
import numpy as np

B, S, D, H, DFF, LN_ = 16, 512, 1024, 8, 4096, 3
DK = D // H
NB = 2
TOK = NB * S
P = 128
ND = D // P      # 8
NQ = S // P      # 4
ISD = 1.0 / float(np.sqrt(DK))
WPAD = 2048

_CACHE = {}


def _build(nlayers=3, taps=(), repeat=1):
    import concourse.bass as bass
    import concourse.mybir as mybir
    from concourse import bacc
    from concourse.tile import TileContext

    dt = mybir.dt
    f32, f32r, bf16, f16, u8, i32 = (dt.float32, dt.float32r, dt.bfloat16,
                                     dt.float16, dt.uint8, dt.int32)
    AF = mybir.ActivationFunctionType
    OP = mybir.AluOpType

    nc = bacc.Bacc(None, target_bir_lowering=False)

    # Every transcendental in this kernel is Exp or Ln. The act-table-load
    # pass picks the first act_info set containing each function, which makes
    # Exp/Ln alternation swap tables every few ops (~2.7us per swap on HW).
    # Steer both to the combined natural_log_exp set by hiding them from the
    # single-function sets (dict identity is the functools.cache singleton;
    # set indices — what walrus consumes — are unchanged).
    from concourse.hw_specs import get_activation_tables
    _tabs = get_activation_tables(nc.m.arch)
    for _name, _fns in _tabs.items():
        if _name != "natural_log_exp_and_others":
            _fns.discard(AF.Exp)
            _fns.discard(AF.Ln)

    def par(name, shape, out=False, dtype=None):
        return nc.declare_dram_parameter(name, list(shape), dtype or f32,
                                         isOutput=out)

    xqa_e = par("xqa", [D, TOK], dtype=f32r)
    xq_e = par("xq", [D, TOK], dtype=f32r)
    kwt_e = par("kwt", [LN_, D, D], dtype=f32r)
    vwt_e = par("vwt", [LN_, D, D], dtype=f32r)
    owt_e = par("owt", [LN_, D, D], dtype=f32r)
    w1t_e = par("w1t", [LN_, D, DFF], dtype=bf16)
    w2t_e = par("w2t", [LN_, DFF, D], dtype=bf16)
    a0f_e = par("a0f", [LN_, H, S]); a1f_e = par("a1f", [LN_, H, S])
    e0f_e = par("e0f", [LN_, H, S]); e1f_e = par("e1f", [LN_, H, S])
    a0r_e = par("a0r", [LN_, H, S]); a1r_e = par("a1r", [LN_, H, S])
    e0r_e = par("e0r", [LN_, H, S]); e1r_e = par("e1r", [LN_, H, S])
    gam_e = par("gam", [1, LN_ * H])
    posn_e = par("posn", [S, S], dtype=f16)
    out_e = par("out", [D, TOK], out=True)
    tap_outs = {}

    with TileContext(nc) as tc:
        pg = tc.alloc_tile_pool(name="glob", bufs=1)
        pdram = tc.alloc_tile_pool(name="dram", bufs=1, space="DRAM")
        psQ = tc.alloc_tile_pool(name="psQ", bufs=2, space="PSUM")
        psT = tc.alloc_tile_pool(name="psT", bufs=1, space="PSUM")
        psAv = tc.alloc_tile_pool(name="psAv", bufs=1, space="PSUM")

        _dmaq = [nc.sync, nc.scalar]
        _dmac = [0]

        def wdma(out, in_):
            eng = _dmaq[_dmac[0] % len(_dmaq)]
            _dmac[0] += 1
            eng.dma_start(out=out, in_=in_)

        def mm_group(psum_ap, pairs):
            n = len(pairs)
            for i, (lt, rh) in enumerate(pairs):
                nc.tensor.matmul(psum_ap, lt, rh,
                                 start=(i == 0), stop=(i == n - 1))

        # ---------------- constants (global pool) ----------------
        ident_f = pg.tile([P, P], f32, name="t", tag="identf")
        nc.gpsimd.memset(ident_f[:], 0.0)
        nc.gpsimd.affine_select(
            out=ident_f[:], in_=ident_f[:], compare_op=OP.not_equal,
            fill=1.0, base=0, channel_multiplier=1, pattern=[[-1, P]])
        ident = pg.tile([P, P], f32r, name="t", tag="ident")
        nc.vector.tensor_copy(ident[:], ident_f[:])

        ones_f = pg.tile([P, 1], f32, name="t", tag="onesf")
        nc.gpsimd.memset(ones_f[:], 1.0)
        ones_col = pg.tile([P, 1], f32r, name="t", tag="ones")
        nc.vector.tensor_copy(ones_col[:], ones_f[:])
        eps5 = pg.tile([P, 1], f32, name="t", tag="eps5")
        nc.gpsimd.memset(eps5[:], 1e-5)

        posn = []
        for qt in range(NQ):
            t = pg.tile([P, S], f16, name="t", tag=f"posn{qt}")
            nc.sync.dma_start(out=t[:], in_=posn_e[qt * P:(qt + 1) * P, :])
            posn.append(t)

        idxt = []
        for h in range(H):
            t = pg.tile([P, 1], i32, name="t", tag=f"idx{h}")
            nc.gpsimd.iota(t[:], pattern=[[1, 1]],
                           base=h * WPAD + (S - 1) - P * (NQ - 1),
                           channel_multiplier=-1)
            idxt.append(t)

        grow = pg.tile([1, LN_ * H], f32, name="t", tag="grow")
        nc.sync.dma_start(out=grow[:], in_=gam_e[:])
        one_c = pg.tile([P, 1], f32, name="t", tag="one_c")
        nc.gpsimd.memset(one_c[:], 1.0)
        # softplus(x) = ln(1 + exp(x)) computed manually (no Softplus table)
        gsp = pg.tile([1, LN_ * H], f32, name="t", tag="gsp")
        nc.scalar.activation(gsp[:], grow[:], AF.Exp)
        nc.scalar.activation(gsp[:], gsp[:], AF.Ln, bias=one_c[:1, :])
        # lgam = ln(softplus(gamma)); te = exp(-exp(0.5*ln(d2)-0.5*ln(r1)+lgam))
        # keeps every ACT call in the natural_log_exp table set (no Sqrt).
        lgam = pg.tile([1, LN_ * H], f32, name="t", tag="lgam")
        nc.scalar.activation(lgam[:], gsp[:], AF.Ln)
        lgam_bc = []
        for i in range(LN_ * H):
            t = pg.tile([P, 1], f32, name="t", tag=f"gbc{i}")
            nc.gpsimd.partition_broadcast(t[:], lgam[0:1, i:i + 1])
            lgam_bc.append(t)

        y_dram = pdram.tile([D, TOK], f32r, name="t", tag="ydram")
        x1_dram = pdram.tile([D, TOK], f32r, name="t", tag="x1dram")

        # ---------------- helpers ----------------
        def dam_prep(l):
            wdam = pdram.tile([1, H * WPAD], u8, name="t", tag="wdam")
            pp = tc.alloc_tile_pool(name=f"dp{l}", bufs=1)

            def half(a0e, a1e, e0e, e1e):
                tA = pp.tile([H, S], f32, name="t", tag="dpA")
                tB = pp.tile([H, S], f32, name="t", tag="dpB")
                tC = pp.tile([H, S], f32, name="t", tag="dpC")
                tD = pp.tile([H, S], f32, name="t", tag="dpD")
                nc.sync.dma_start(out=tA[:], in_=e0e[l])
                nc.sync.dma_start(out=tB[:], in_=e1e[l])
                nc.scalar.activation(tA[:], tA[:], AF.Ln, bias=eps5[:H, :])
                nc.scalar.activation(tB[:], tB[:], AF.Ln, bias=eps5[:H, :])
                nc.vector.tensor_tensor(tA[:], tA[:], tB[:], OP.subtract)
                nc.sync.dma_start(out=tC[:], in_=a1e[l])
                nc.sync.dma_start(out=tD[:], in_=a0e[l])
                nc.vector.tensor_tensor(tC[:], tC[:], tD[:], OP.subtract)
                nc.vector.tensor_tensor(tA[:], tA[:], tC[:], OP.add)
                c = pp.tile([H, S], u8, name="t", tag="dpc", bufs=2)
                nc.vector.tensor_scalar(c[:], tA[:], 0.0, None, OP.is_gt)
                return c

            cf = half(a0f_e, a1f_e, e0f_e, e1f_e)
            cr = half(a0r_e, a1r_e, e0r_e, e1r_e)
            dst_r = bass.AP(tensor=wdam.tensor, offset=0,
                            ap=[[WPAD, H], [1, S - 1]])
            dst_f = bass.AP(tensor=wdam.tensor, offset=S - 1,
                            ap=[[WPAD, H], [1, S]])
            nc.sync.dma_start(out=dst_r, in_=cr[:, 0:S - 1])
            nc.sync.dma_start(out=dst_f, in_=cf[:])
            pp.release()
            return wdam

        def layernorm(pool, r_t, dsts):
            """r_t: 8 [P,S] f32r tiles; writes (x-mu)/sigma into dsts APs."""
            s1 = psT.tile([1, S], f32, name="t", tag="pt0")
            mm_group(s1[:], [(ones_col[:], r_t[od][:]) for od in range(ND)])
            s2 = psT.tile([1, S], f32, name="t", tag="pt1")
            for od in range(ND):
                sq = pool.tile([P, S], f32r, name="t", tag="sqtmp", bufs=1)
                nc.vector.tensor_tensor(sq[:], r_t[od][:], r_t[od][:],
                                        OP.mult)
                nc.tensor.matmul(s2[:], ones_col[:], sq[:],
                                 start=(od == 0), stop=(od == ND - 1))
            mean = pool.tile([1, S], f32, name="t", tag="lnr0", bufs=1)
            nc.vector.tensor_scalar(mean[:], s1[:], 1.0 / D, None, OP.mult)
            msq = pool.tile([1, S], f32, name="t", tag="lnr1", bufs=1)
            nc.vector.tensor_scalar(msq[:], s2[:], 1.0 / D, None, OP.mult)
            m2 = pool.tile([1, S], f32, name="t", tag="lnr2", bufs=1)
            nc.vector.tensor_tensor(m2[:], mean[:], mean[:], OP.mult)
            nc.vector.tensor_tensor(msq[:], msq[:], m2[:], OP.subtract)
            # rstd = exp(-0.5*ln(var+eps)) — stays in the ln/exp table set
            nc.scalar.activation(msq[:], msq[:], AF.Ln, bias=eps5[:1, :])
            nc.scalar.activation(m2[:], msq[:], AF.Exp, scale=-0.5)
            nc.vector.tensor_scalar(mean[:], mean[:], -1.0, None, OP.mult)
            nc.vector.tensor_tensor(mean[:], mean[:], m2[:], OP.mult)
            Ab = pool.tile([P, S], f32, name="t", tag="Ab", bufs=1)
            nc.gpsimd.partition_broadcast(Ab[:], m2[:])
            Cb = pool.tile([P, S], f32, name="t", tag="Cb", bufs=1)
            nc.gpsimd.partition_broadcast(Cb[:], mean[:])
            for od in range(ND):
                t1 = pool.tile([P, S], f32, name="t", tag="lnt", bufs=1)
                nc.vector.tensor_tensor(t1[:], r_t[od][:], Ab[:], OP.mult)
                nc.gpsimd.tensor_tensor(dsts[od], t1[:], Cb[:], OP.add)

        def attention_head(pool, l, bmask, h, K, V, att_dst, damG):
            pst = [psT.tile([P, S], f32r, name="t", tag=f"pt{kc}")
                   for kc in range(NQ)]
            ktile = K[h]
            for qt in range(NQ):
                w = P * (qt + 1)
                ps = psQ.tile([P, S], f32, name="t", tag="qk")
                mm_group(ps[:], [(ktile[:, qt * P:qt * P + P], ktile[:])])
                doff = P * (NQ - 1) - P * qt
                e1 = pool.tile([P, S], f32, name="t", tag="e1")
                nc.scalar.activation(e1[:], ps[:], AF.Exp, scale=ISD)
                e1c = pool.tile([P, S], f32, name="t", tag="tmpA", bufs=4)
                nc.gpsimd.affine_select(
                    out=e1c[:, :w], in_=e1[:, :w], compare_op=OP.is_gt,
                    fill=0.0, base=qt * P + bmask, channel_multiplier=1,
                    pattern=[[-1, w]])
                r1 = pool.tile([P, 1], f32, name="t", tag="sm_r1")
                nc.vector.scalar_tensor_tensor(
                    e1[:], e1[:], 1.0, damG[:, doff:doff + S],
                    OP.mult, OP.mult, accum_out=r1[:])
                cum = pool.tile([P, S], f32, name="t", tag="tmpB", bufs=3)
                nc.vector.tensor_tensor_scan(
                    cum[:, :w], e1c[:, :w], e1c[:, :w], 0.0, OP.add, OP.bypass)
                lnr1 = pool.tile([P, 1], f32, name="t", tag="sm_rc1")
                nc.scalar.activation(lnr1[:], r1[:], AF.Ln)
                brow = pool.tile([P, 1], f32, name="t", tag="sm_brow")
                nc.vector.scalar_tensor_tensor(
                    brow[:], lnr1[:], -0.5, lgam_bc[l * H + h][:],
                    OP.mult, OP.add)
                d2 = pool.tile([P, S], f32, name="t", tag="tmpA", bufs=4)
                nc.vector.scalar_tensor_tensor(
                    d2[:, :w], cum[:, :w], cum[:, w - 1:w], posn[qt][:, :w],
                    OP.subtract, OP.mult)
                dist = pool.tile([P, S], f32, name="t", tag="tmpB", bufs=3)
                nc.scalar.activation(dist[:, :w], d2[:, :w], AF.Ln)
                sga = pool.tile([P, S], f32, name="t", tag="tmpA", bufs=4)
                nc.scalar.activation(sga[:, :w], dist[:, :w], AF.Exp,
                                     scale=0.5, bias=brow[:])
                te = pool.tile([P, S], f32, name="t", tag="tmpB", bufs=3)
                nc.scalar.activation(te[:, :w], sga[:, :w], AF.Exp,
                                     scale=-1.0)
                t2u = pool.tile([P, S], f32, name="t", tag="tmpA", bufs=4)
                nc.vector.scalar_tensor_tensor(
                    t2u[:, :w], te[:, :w], 1e-5, ps[:, :w], OP.max, OP.mult)
                t2m = pool.tile([P, S], f32, name="t", tag="tmpB", bufs=3)
                nc.gpsimd.affine_select(
                    out=t2m[:, :w], in_=t2u[:, :w], compare_op=OP.is_gt,
                    fill=-1e30, base=qt * P + bmask, channel_multiplier=1,
                    pattern=[[-1, w]])
                e2 = pool.tile([P, S], f32, name="t", tag="tmpA", bufs=4)
                r2 = pool.tile([P, 1], f32, name="t", tag="sm_r2")
                nc.scalar.activation(e2[:, :w], t2m[:, :w], AF.Exp,
                                     scale=ISD, accum_out=r2[:])
                nc.vector.tensor_scalar(r2[:], r2[:], 1e-30, None, OP.max)
                rec2 = pool.tile([P, 1], f32, name="t", tag="sm_rc2")
                nc.vector.reciprocal(rec2[:], r2[:])
                pr = pool.tile([P, S], f32r, name="t", tag="probs", bufs=2)
                nc.vector.tensor_scalar(pr[:, :w], e2[:, :w], rec2[:],
                                        None, OP.mult)
                for kc in range(qt + 1):
                    nc.tensor.transpose(
                        pst[kc][:, qt * P:qt * P + P],
                        pr[:, kc * P:kc * P + P], ident[:])
            prT = []
            for kc in range(NQ):
                t = pool.tile([P, S], f32r, name="t", tag=f"prT{kc}", bufs=1)
                nc.vector.tensor_copy(t[:, kc * P:], pst[kc][:, kc * P:])
                prT.append(t)
            pav = psAv.tile([P, S], f32, name="t", tag="av")
            for kc in range(NQ):
                nc.tensor.matmul(
                    pav[:, kc * P:], V[kc][:, h * DK:(h + 1) * DK],
                    prT[kc][:, kc * P:],
                    start=(kc == 0), stop=(kc == NQ - 1))
            nc.vector.tensor_copy(att_dst, pav[:])

        def layer(l, bmask, apply_pos, xq_src, vals_src, out_dram,
                  final=False):
            """xq_src: 8 [P,TOK] f32r tiles (query/key input).
            vals_src: 'self' or a DRAM tile to stream per b.
            out_dram: DRAM target AP base for the layer output."""
            wdam = dam_prep(l)
            pdam = tc.alloc_tile_pool(name=f"dam{l}", bufs=1)
            damGs = []
            for h in range(H):
                g = pdam.tile([P, 2 * S - 1], u8, name="t", tag=f"damG{h}")
                nc.gpsimd.indirect_dma_start(
                    out=g[:], out_offset=None, in_=wdam[:],
                    in_offset=bass.IndirectOffsetOnAxis(
                        ap=idxt[h][:, :1], axis=1))
                damGs.append(g)
            for b in range(NB):
                bs = b * S
                pool = tc.alloc_tile_pool(name=f"att{l}{b}", bufs=2)
                # ---- K projection (q==k), kwt streamed in od-halves
                K = []
                for half in range(2):
                    wk = []
                    for idt in range(ND):
                        t = pool.tile([P, S], f32r, name="t", tag=f"wbig{idt}",
                                          bufs=2)
                        wdma(
                            t[:],
                            kwt_e[l, idt * P:(idt + 1) * P,
                                      half * S:(half + 1) * S])
                        wk.append(t)
                    for oc in range(4):
                        od = half * 4 + oc
                        ps = psQ.tile([P, S], f32, name="t", tag="qk")
                        mm_group(ps[:], [
                            (wk[idt][:, oc * P:(oc + 1) * P],
                             xq_src[idt][:, bs:bs + S]) for idt in range(ND)])
                        kt = pool.tile([P, S], f32r, name="t", tag=f"K{od}",
                                       bufs=1)
                        nc.vector.tensor_copy(kt[:], ps[:])
                        K.append(kt)
                # ---- VALS for v-projection
                if vals_src == "self":
                    vals = [xq_src[idt][:, bs:bs + S] for idt in range(ND)]
                else:
                    vt = []
                    for idt in range(ND):
                        t = pool.tile([P, S], f32r, name="t", tag=f"att{idt}", bufs=1)
                        wdma(
                            t[:],
                            vals_src[idt * P:(idt + 1) * P, bs:bs + S])
                        vt.append(t)
                    vals = [t[:] for t in vt]
                # ---- V projection (token-major), vwt streamed in d-halves
                V = [pool.tile([P, D], f32r, name="t", tag=f"V{st}", bufs=1)
                     for st in range(NQ)]
                for half in range(2):
                    wv = []
                    for idt in range(ND):
                        t = pool.tile([P, S], f32r, name="t", tag=f"wbig{idt}",
                                          bufs=2)
                        wdma(
                            t[:],
                            vwt_e[l, idt * P:(idt + 1) * P,
                                      half * S:(half + 1) * S])
                        wv.append(t)
                    for st in range(NQ):
                        ps = psQ.tile([P, S], f32, name="t", tag="qk")
                        mm_group(ps[:], [
                            (vals[idt][:, st * P:(st + 1) * P], wv[idt][:])
                            for idt in range(ND)])
                        nc.vector.tensor_copy(
                            V[st][:, half * S:(half + 1) * S], ps[:])
                # ---- attention heads
                att = [pool.tile([P, S], f32r, name="t", tag=f"att{od}", bufs=1)
                       for od in range(ND)]
                for h in range(H):
                    attention_head(pool, l, bmask, h, K, V, att[h][:], damGs[h])
                # ---- o-projection + residual, owt streamed in od-halves
                r_t = []
                for half in range(2):
                    wo = []
                    for idt in range(ND):
                        t = pool.tile([P, S], f32r, name="t", tag=f"wbig{idt}",
                                          bufs=2)
                        wdma(
                            t[:],
                            owt_e[l, idt * P:(idt + 1) * P,
                                      half * S:(half + 1) * S])
                        wo.append(t)
                    for oc in range(4):
                        od = half * 4 + oc
                        ps = psQ.tile([P, S], f32, name="t", tag="qk")
                        mm_group(ps[:], [
                            (wo[idt][:, oc * P:(oc + 1) * P], att[idt][:])
                            for idt in range(ND)])
                        rt = pool.tile([P, S], f32r, name="t", tag=f"r{od}",
                                       bufs=1)
                        nc.vector.tensor_tensor(
                            rt[:], xq_src[od][:, bs:bs + S], ps[:], OP.add)
                        r_t.append(rt)
                # ---- LN1
                if apply_pos:
                    xp = [pg.tile([P, S], f32r, name="t", tag=f"xp{od}")
                          for od in range(ND)]
                    layernorm(pool, r_t, [t[:] for t in xp])
                else:
                    ot = [pool.tile([P, S], f32 if final else f32r, name="t",
                                    tag="outt", bufs=2)
                          for _ in range(ND)]
                    layernorm(pool, r_t, [t[:] for t in ot])
                    for od in range(ND):
                        nc.sync.dma_start(
                            out=out_dram[od * P:(od + 1) * P, bs:bs + S],
                            in_=ot[od][:])
                pool.release()

                if not apply_pos:
                    continue
                # ---- FFN + LN2
                fp = tc.alloc_tile_pool(name=f"ffn{l}{b}", bufs=2)
                xpb = []
                for od in range(ND):
                    t = fp.tile([P, S], bf16, name="t", tag=f"xpb{od}", bufs=1)
                    nc.vector.tensor_copy(t[:], xp[od][:])
                    xpb.append(t)
                h1 = []
                for fc in range(8):
                    w1c = []
                    for idt in range(ND):
                        t = fp.tile([P, S], bf16, name="t", tag=f"w1c{idt}")
                        wdma(
                            t[:],
                            w1t_e[l, idt * P:(idt + 1) * P,
                                      fc * S:(fc + 1) * S])
                        w1c.append(t)
                    for fl in range(4):
                        ps = psQ.tile([P, S], f32, name="t", tag="qk")
                        mm_group(ps[:], [
                            (w1c[idt][:, fl * P:(fl + 1) * P], xpb[idt][:])
                            for idt in range(ND)])
                        ht = fp.tile([P, S], bf16, name="t",
                                     tag=f"h1_{fc * 4 + fl}", bufs=1)
                        nc.vector.tensor_scalar(ht[:], ps[:], 0.0, None,
                                                OP.max)
                        h1.append(ht)
                r_t = []
                for og in range(2):
                    pso = [psT.tile([P, S], f32, name="t", tag=f"pt{oc}")
                           for oc in range(4)]
                    for fc in range(8):
                        w2c = []
                        for fl in range(4):
                            ft = fc * 4 + fl
                            t = fp.tile([P, S], bf16, name="t", tag=f"w2c{fl}")
                            wdma(
                                t[:],
                                w2t_e[l, ft * P:(ft + 1) * P,
                                          og * S:(og + 1) * S])
                            w2c.append(t)
                        for fl in range(4):
                            ft = fc * 4 + fl
                            for oc in range(4):
                                nc.tensor.matmul(
                                    pso[oc][:],
                                    w2c[fl][:, oc * P:(oc + 1) * P],
                                    h1[ft][:],
                                    start=(fc == 0 and fl == 0),
                                    stop=(fc == 7 and fl == 3))
                    for oc in range(4):
                        od = og * 4 + oc
                        rt = fp.tile([P, S], f32r, name="t", tag=f"r{od}",
                                     bufs=1)
                        nc.vector.tensor_tensor(
                            rt[:], xp[od][:], pso[oc][:], OP.add)
                        r_t.append(rt)
                ot = [fp.tile([P, S], f32 if final else f32r, name="t",
                              tag="outt", bufs=4)
                      for _ in range(ND)]
                layernorm(fp, r_t, [t[:] for t in ot])
                for od in range(ND):
                    nc.sync.dma_start(
                        out=out_dram[od * P:(od + 1) * P, bs:bs + S],
                        in_=ot[od][:])
                fp.release()
            pdam.release()

        def load_x(src):
            tiles = []
            for od in range(ND):
                t = pg.tile([P, TOK], f32r, name="t", tag=f"xa{od}")
                nc.sync.dma_start(out=t[:], in_=src[od * P:(od + 1) * P, :])
                tiles.append(t)
            return tiles

        # ================= driver =================
        for _rep in range(repeat):
            XA = load_x(xqa_e)
            layer(0, 1, True, XA, "self", y_dram)
            if nlayers >= 2:
                XA = load_x(xq_e)
                layer(1, 1, False, XA, "self", x1_dram)
            if nlayers >= 3:
                XA = load_x(x1_dram)
                layer(2, 0, True, XA, y_dram, out_e, final=True)
            if nlayers == 1:
                nc.gpsimd.dma_start(out=out_e[:], in_=y_dram[:])
            elif nlayers == 2:
                nc.gpsimd.dma_start(out=out_e[:], in_=x1_dram[:])

        psAv.release()
        psT.release()
        psQ.release()
        pdram.release()
        pg.release()

    nc.finalize()
    return nc, tap_outs


def _get_nc(nlayers=3, taps=(), repeat=1):
    key = (nlayers, tuple(sorted(taps)), repeat)
    if key not in _CACHE:
        _CACHE[key] = _build(nlayers, taps, repeat)
    return _CACHE[key]


def _make_in_maps(inputs):
    qa = np.asarray(inputs["qa_embed_data"])
    qd = np.asarray(inputs["q_embed_data"])
    al = np.asarray(inputs["alphas"])
    ge = np.asarray(inputs["gumbel_E"])
    a0f = al[..., 0]; a1f = al[..., 1]
    e0f = ge[..., 0]; e1f = ge[..., 1]
    i_ = np.arange(S)
    shared = {
        "kwt": np.asarray(inputs["kW"]).transpose(0, 2, 1),
        "vwt": np.asarray(inputs["vW"]).transpose(0, 2, 1),
        "owt": np.asarray(inputs["oW"]).transpose(0, 2, 1),
        "w1t": np.asarray(inputs["w1"]).transpose(0, 2, 1),
        "w2t": np.asarray(inputs["w2"]).transpose(0, 2, 1),
        "a0f": a0f, "a1f": a1f, "e0f": e0f, "e1f": e1f,
        "a0r": a0f[:, :, ::-1], "a1r": a1f[:, :, ::-1],
        "e0r": e0f[:, :, ::-1], "e1r": e1f[:, :, ::-1],
        "gam": np.asarray(inputs["gammas"]).reshape(1, LN_ * H),
        "posn": -np.abs(i_[:, None] - i_[None, :]),
    }
    import ml_dtypes
    casts = {"w1t": ml_dtypes.bfloat16, "w2t": ml_dtypes.bfloat16,
             "posn": np.float16}
    shared = {k: np.ascontiguousarray(v, dtype=casts.get(k, np.float32))
              for k, v in shared.items()}

    def feat_major(x, c):
        pair = np.asarray(x[NB * c:NB * c + NB])        # [2, S, D]
        return np.ascontiguousarray(
            pair.transpose(2, 0, 1).reshape(D, TOK), dtype=np.float32)

    in_maps = []
    for c in range(8):
        m = dict(shared)
        m["xqa"] = feat_major(qa, c)
        m["xq"] = feat_major(qd, c)
        in_maps.append(m)
    return in_maps


def _gather_out(results):
    outs = []
    for r in results:
        o = r["out"].reshape(D, NB, S).transpose(1, 2, 0)
        outs.append(o)
    return np.ascontiguousarray(np.concatenate(outs, axis=0))


def kernel(**inputs):
    from concourse.bass_utils import run_bass_kernel_spmd
    nc, _ = _get_nc()
    in_maps = _make_in_maps(inputs)
    res = run_bass_kernel_spmd(nc, in_maps, core_ids=list(range(8)))
    return _gather_out(res.results)



# revision 19
# speedup vs baseline: 1.1973x; 1.1350x over previous
"""Trainium2 Bass kernel for nn_Architecture_50629074485965 (3-layer AKT-style
transformer, B=16 S=512 D=1024 H=8 DFF=4096).

Sharding: data-parallel over batch — 2 batches per core, 8 cores, no
collectives.  Activations are feature-major [D on partitions, tokens free] so
every matmul chains without activation transposes (weights host-pre-
transposed).  Matmuls run in float32r (TF32-like, ~1.6e-4 rel err, 4x fp32
rate).  FFN hidden + w2 in bf16.  Layer outputs bounce through DRAM.

The problem spec pins all biases to zeros and LN affines to identity, so those
terms are skipped.

Attention per (b,h), per 128-row q-tile (q-major [q, k] layout):
  psum  = q @ k^T                         (PE f32r)
  e1    = Exp(psum/sqrt(dk))              (ACT, full width)
  e1c   = causal(e1)                      (GPSIMD affine_select, width w)
  r1    = sum_j e1*dam01                  (DVE stt accum, e1 in-place;
                                           dam01 = u8 [128,512] row-window
                                           gather from a per-head Toeplitz
                                           vector via indirect_dma_start)
  cum   = cumsum(e1c)                     (DVE tensor_tensor_scan)
  d2    = (cum - rowtot) * (-|i-j|) >= 0  (DVE stt, posn = -|i-j| in f16)
  dist  = Sqrt(d2 * (1/r1))               (ACT, scale AP)
  te    = Exp(dist * -softplus(gamma))    (ACT, scale AP)
  t2u   = max(te,1e-5) * psum             (DVE stt)
  t2m   = causal(t2u, fill=-1e30)         (GPSIMD affine_select)
  e2,r2 = Exp(t2m/sqrt(dk)) + row-sum     (ACT accum_out)
  probs = e2 * (1/max(r2,1e-30)) -> f32r  (DVE)
  probsT blocks: PE transpose -> psum -> sbuf (ACT copies)
  att   = v-chunks(lhsT) @ probsT -> feature-major  (PE)
"""
import sys
sys.path.insert(0, "/opt/trn_rl_repo")
import numpy as np

B, S, D, H, DFF, LN_ = 16, 512, 1024, 8, 4096, 3
DK = D // H
NB = 2
TOK = NB * S
P = 128
ND = D // P      # 8
NQ = S // P      # 4
ISD = 1.0 / float(np.sqrt(DK))
WPAD = 2048

_CACHE = {}


def _build(nlayers=3, taps=(), repeat=1):
    import concourse.bass as bass
    import concourse.mybir as mybir
    from concourse import bacc
    from concourse.tile import TileContext

    dt = mybir.dt
    f32, f32r, bf16, f16, u8, i32 = (dt.float32, dt.float32r, dt.bfloat16,
                                     dt.float16, dt.uint8, dt.int32)
    AF = mybir.ActivationFunctionType
    OP = mybir.AluOpType

    nc = bacc.Bacc(None, target_bir_lowering=False)

    # Every transcendental in this kernel is Exp or Ln. The act-table-load
    # pass picks the first act_info set containing each function, which makes
    # Exp/Ln alternation swap tables every few ops (~2.7us per swap on HW).
    # Steer both to the combined natural_log_exp set by hiding them from the
    # single-function sets (dict identity is the functools.cache singleton;
    # set indices — what walrus consumes — are unchanged).
    from concourse.hw_specs import get_activation_tables
    _tabs = get_activation_tables(nc.m.arch)
    for _name, _fns in _tabs.items():
        if _name != "natural_log_exp_and_others":
            _fns.discard(AF.Exp)
            _fns.discard(AF.Ln)

    def par(name, shape, out=False, dtype=None):
        return nc.declare_dram_parameter(name, list(shape), dtype or f32,
                                         isOutput=out)

    xqa_e = par("xqa", [D, TOK], dtype=f32r)
    xq_e = par("xq", [D, TOK], dtype=f32r)
    kwt_e = par("kwt", [LN_, D, D], dtype=f32r)
    vwt_e = par("vwt", [LN_, D, D], dtype=f32r)
    owt_e = par("owt", [LN_, D, D], dtype=f32r)
    w1t_e = par("w1t", [LN_, D, DFF], dtype=bf16)
    w2t_e = par("w2t", [LN_, DFF, D], dtype=bf16)
    a0f_e = par("a0f", [LN_, H, S]); a1f_e = par("a1f", [LN_, H, S])
    e0f_e = par("e0f", [LN_, H, S]); e1f_e = par("e1f", [LN_, H, S])
    a0r_e = par("a0r", [LN_, H, S]); a1r_e = par("a1r", [LN_, H, S])
    e0r_e = par("e0r", [LN_, H, S]); e1r_e = par("e1r", [LN_, H, S])
    gam_e = par("gam", [1, LN_ * H])
    posn_e = par("posn", [S, S], dtype=f16)
    out_e = par("out", [D, TOK], out=True)
    tap_outs = {}

    with TileContext(nc) as tc:
        pg = tc.alloc_tile_pool(name="glob", bufs=1)
        pdram = tc.alloc_tile_pool(name="dram", bufs=1, space="DRAM")
        psQ = tc.alloc_tile_pool(name="psQ", bufs=3, space="PSUM")
        psT = tc.alloc_tile_pool(name="psT", bufs=1, space="PSUM")
        psAv = tc.alloc_tile_pool(name="psAv", bufs=1, space="PSUM")

        _dmaq = [nc.sync, nc.scalar]
        _dmac = [0]

        def wdma(out, in_):
            eng = _dmaq[_dmac[0] % len(_dmaq)]
            _dmac[0] += 1
            eng.dma_start(out=out, in_=in_)

        def mm_group(psum_ap, pairs):
            n = len(pairs)
            for i, (lt, rh) in enumerate(pairs):
                nc.tensor.matmul(psum_ap, lt, rh,
                                 start=(i == 0), stop=(i == n - 1))

        # ---------------- constants (global pool) ----------------
        ident_f = pg.tile([P, P], f32, name="t", tag="identf")
        nc.gpsimd.memset(ident_f[:], 0.0)
        nc.gpsimd.affine_select(
            out=ident_f[:], in_=ident_f[:], compare_op=OP.not_equal,
            fill=1.0, base=0, channel_multiplier=1, pattern=[[-1, P]])
        ident = pg.tile([P, P], f32r, name="t", tag="ident")
        nc.vector.tensor_copy(ident[:], ident_f[:])

        ones_f = pg.tile([P, 1], f32, name="t", tag="onesf")
        nc.gpsimd.memset(ones_f[:], 1.0)
        ones_col = pg.tile([P, 1], f32r, name="t", tag="ones")
        nc.vector.tensor_copy(ones_col[:], ones_f[:])
        eps5 = pg.tile([P, 1], f32, name="t", tag="eps5")
        nc.gpsimd.memset(eps5[:], 1e-5)

        posn = []
        for qt in range(NQ):
            t = pg.tile([P, S], f16, name="t", tag=f"posn{qt}")
            nc.sync.dma_start(out=t[:], in_=posn_e[qt * P:(qt + 1) * P, :])
            posn.append(t)

        idxt = []
        for h in range(H):
            t = pg.tile([P, 1], i32, name="t", tag=f"idx{h}")
            nc.gpsimd.iota(t[:], pattern=[[1, 1]],
                           base=h * WPAD + (S - 1) - P * (NQ - 1),
                           channel_multiplier=-1)
            idxt.append(t)

        grow = pg.tile([1, LN_ * H], f32, name="t", tag="grow")
        nc.sync.dma_start(out=grow[:], in_=gam_e[:])
        one_c = pg.tile([P, 1], f32, name="t", tag="one_c")
        nc.gpsimd.memset(one_c[:], 1.0)
        # softplus(x) = ln(1 + exp(x)) computed manually (no Softplus table)
        gsp = pg.tile([1, LN_ * H], f32, name="t", tag="gsp")
        nc.scalar.activation(gsp[:], grow[:], AF.Exp)
        nc.scalar.activation(gsp[:], gsp[:], AF.Ln, bias=one_c[:1, :])
        # lgam = ln(softplus(gamma)); te = exp(-exp(0.5*ln(d2)-0.5*ln(r1)+lgam))
        # keeps every ACT call in the natural_log_exp table set (no Sqrt).
        lgam = pg.tile([1, LN_ * H], f32, name="t", tag="lgam")
        nc.scalar.activation(lgam[:], gsp[:], AF.Ln)
        lgam_bc = []
        for i in range(LN_ * H):
            t = pg.tile([P, 1], f32, name="t", tag=f"gbc{i}")
            nc.gpsimd.partition_broadcast(t[:], lgam[0:1, i:i + 1])
            lgam_bc.append(t)

        y_dram = pdram.tile([D, TOK], f32r, name="t", tag="ydram")
        x1_dram = pdram.tile([D, TOK], f32r, name="t", tag="x1dram")

        # ---------------- helpers ----------------
        def dam_prep(l):
            wdam = pdram.tile([1, H * WPAD], u8, name="t", tag="wdam")
            pp = tc.alloc_tile_pool(name=f"dp{l}", bufs=1)

            def half(a0e, a1e, e0e, e1e):
                tA = pp.tile([H, S], f32, name="t", tag="dpA")
                tB = pp.tile([H, S], f32, name="t", tag="dpB")
                tC = pp.tile([H, S], f32, name="t", tag="dpC")
                tD = pp.tile([H, S], f32, name="t", tag="dpD")
                nc.sync.dma_start(out=tA[:], in_=e0e[l])
                nc.sync.dma_start(out=tB[:], in_=e1e[l])
                nc.scalar.activation(tA[:], tA[:], AF.Ln, bias=eps5[:H, :])
                nc.scalar.activation(tB[:], tB[:], AF.Ln, bias=eps5[:H, :])
                nc.vector.tensor_tensor(tA[:], tA[:], tB[:], OP.subtract)
                nc.sync.dma_start(out=tC[:], in_=a1e[l])
                nc.sync.dma_start(out=tD[:], in_=a0e[l])
                nc.vector.tensor_tensor(tC[:], tC[:], tD[:], OP.subtract)
                nc.vector.tensor_tensor(tA[:], tA[:], tC[:], OP.add)
                c = pp.tile([H, S], u8, name="t", tag="dpc", bufs=2)
                nc.vector.tensor_scalar(c[:], tA[:], 0.0, None, OP.is_gt)
                return c

            cf = half(a0f_e, a1f_e, e0f_e, e1f_e)
            cr = half(a0r_e, a1r_e, e0r_e, e1r_e)
            dst_r = bass.AP(tensor=wdam.tensor, offset=0,
                            ap=[[WPAD, H], [1, S - 1]])
            dst_f = bass.AP(tensor=wdam.tensor, offset=S - 1,
                            ap=[[WPAD, H], [1, S]])
            nc.sync.dma_start(out=dst_r, in_=cr[:, 0:S - 1])
            nc.sync.dma_start(out=dst_f, in_=cf[:])
            pp.release()
            return wdam

        def layernorm(pool, r_t, dsts):
            """r_t: 8 [P,S] f32r tiles; writes (x-mu)/sigma into dsts APs."""
            s1 = psT.tile([1, S], f32, name="t", tag="pt0")
            mm_group(s1[:], [(ones_col[:], r_t[od][:]) for od in range(ND)])
            s2 = psT.tile([1, S], f32, name="t", tag="pt1")
            for od in range(ND):
                sq = pool.tile([P, S], f32r, name="t", tag="tmpA", bufs=6)
                nc.vector.tensor_tensor(sq[:], r_t[od][:], r_t[od][:],
                                        OP.mult)
                nc.tensor.matmul(s2[:], ones_col[:], sq[:],
                                 start=(od == 0), stop=(od == ND - 1))
            mean = pool.tile([1, S], f32, name="t", tag="lnr0", bufs=1)
            nc.vector.tensor_scalar(mean[:], s1[:], 1.0 / D, None, OP.mult)
            msq = pool.tile([1, S], f32, name="t", tag="lnr1", bufs=1)
            nc.vector.tensor_scalar(msq[:], s2[:], 1.0 / D, None, OP.mult)
            m2 = pool.tile([1, S], f32, name="t", tag="lnr2", bufs=1)
            nc.vector.tensor_tensor(m2[:], mean[:], mean[:], OP.mult)
            nc.vector.tensor_tensor(msq[:], msq[:], m2[:], OP.subtract)
            # rstd = exp(-0.5*ln(var+eps)) — stays in the ln/exp table set
            nc.scalar.activation(msq[:], msq[:], AF.Ln, bias=eps5[:1, :])
            nc.scalar.activation(m2[:], msq[:], AF.Exp, scale=-0.5)
            nc.vector.tensor_scalar(mean[:], mean[:], -1.0, None, OP.mult)
            nc.vector.tensor_tensor(mean[:], mean[:], m2[:], OP.mult)
            Ab = pool.tile([P, S], f32, name="t", tag="Ab", bufs=1)
            nc.gpsimd.partition_broadcast(Ab[:], m2[:])
            Cb = pool.tile([P, S], f32, name="t", tag="Cb", bufs=1)
            nc.gpsimd.partition_broadcast(Cb[:], mean[:])
            for od in range(ND):
                t1 = pool.tile([P, S], f32, name="t", tag="lnt", bufs=1)
                nc.vector.tensor_tensor(t1[:], r_t[od][:], Ab[:], OP.mult)
                nc.gpsimd.tensor_tensor(dsts[od], t1[:], Cb[:], OP.add)

        def attention_head(pool, l, bmask, h, K, V, att_dst, damG):
            pst = [psT.tile([P, S], f32r, name="t", tag=f"pt{kc}")
                   for kc in range(NQ)]
            ktile = K[h]
            for qt in range(NQ):
                w = P * (qt + 1)
                ps = psQ.tile([P, S], f32, name="t", tag="qk")
                mm_group(ps[:], [(ktile[:, qt * P:qt * P + P], ktile[:])])
                doff = P * (NQ - 1) - P * qt
                e1 = pool.tile([P, S], f32, name="t", tag="e1", bufs=3)
                nc.scalar.activation(e1[:], ps[:], AF.Exp, scale=ISD)
                # lne = ln(e1) = ps*ISD reconstructs the scaled scores so the
                # PSUM bank frees right after the Exp instead of being held
                # through the whole chain (psQ is the chain-parallelism cap).
                lne = pool.tile([P, S], f32, name="t", tag="tmpA", bufs=6)
                nc.scalar.activation(lne[:, :w], e1[:, :w], AF.Ln)
                e1c = pool.tile([P, S], f32, name="t", tag="tmpA", bufs=6)
                nc.gpsimd.affine_select(
                    out=e1c[:, :w], in_=e1[:, :w], compare_op=OP.is_gt,
                    fill=0.0, base=qt * P + bmask, channel_multiplier=1,
                    pattern=[[-1, w]])
                r1 = pool.tile([P, 1], f32, name="t", tag="sm_r1")
                edam = pool.tile([P, S], bf16, name="t", tag="edam", bufs=2)
                nc.vector.scalar_tensor_tensor(
                    edam[:], e1[:], 1.0, damG[:, doff:doff + S],
                    OP.mult, OP.mult, accum_out=r1[:])
                cum = pool.tile([P, S], f32, name="t", tag="tmpB", bufs=6)
                nc.vector.tensor_tensor_scan(
                    cum[:, :w], e1c[:, :w], e1c[:, :w], 0.0, OP.add, OP.bypass)
                lnr1 = pool.tile([P, 1], f32, name="t", tag="sm_rc1")
                nc.scalar.activation(lnr1[:], r1[:], AF.Ln)
                brow = pool.tile([P, 1], f32, name="t", tag="sm_brow")
                nc.vector.scalar_tensor_tensor(
                    brow[:], lnr1[:], -0.5, lgam_bc[l * H + h][:],
                    OP.mult, OP.add)
                d2 = pool.tile([P, S], f32, name="t", tag="tmpA", bufs=6)
                nc.vector.scalar_tensor_tensor(
                    d2[:, :w], cum[:, :w], cum[:, w - 1:w], posn[qt][:, :w],
                    OP.subtract, OP.mult)
                dist = pool.tile([P, S], f32, name="t", tag="tmpB", bufs=6)
                nc.scalar.activation(dist[:, :w], d2[:, :w], AF.Ln)
                sga = pool.tile([P, S], f32, name="t", tag="tmpA", bufs=6)
                nc.scalar.activation(sga[:, :w], dist[:, :w], AF.Exp,
                                     scale=0.5, bias=brow[:])
                te = pool.tile([P, S], f32, name="t", tag="tmpB", bufs=6)
                nc.scalar.activation(te[:, :w], sga[:, :w], AF.Exp,
                                     scale=-1.0)
                t2u = pool.tile([P, S], f32, name="t", tag="tmpA", bufs=6)
                nc.vector.scalar_tensor_tensor(
                    t2u[:, :w], te[:, :w], 1e-5, lne[:, :w], OP.max, OP.mult)
                # causal boundary only cuts the 128-wide diagonal block;
                # mask it in place instead of re-writing the full width.
                nc.gpsimd.affine_select(
                    out=t2u[:, w - P:w], in_=t2u[:, w - P:w],
                    compare_op=OP.is_gt, fill=-1e30, base=bmask,
                    channel_multiplier=1, pattern=[[-1, P]])
                e2 = pool.tile([P, S], f32, name="t", tag="tmpB", bufs=6)
                r2 = pool.tile([P, 1], f32, name="t", tag="sm_r2")
                nc.scalar.activation(e2[:, :w], t2u[:, :w], AF.Exp,
                                     accum_out=r2[:])
                nc.vector.tensor_scalar(r2[:], r2[:], 1e-30, None, OP.max)
                rec2 = pool.tile([P, 1], f32, name="t", tag="sm_rc2")
                nc.vector.reciprocal(rec2[:], r2[:])
                pr = pool.tile([P, S], f32r, name="t", tag="probs", bufs=3)
                nc.vector.tensor_scalar(pr[:, :w], e2[:, :w], rec2[:],
                                        None, OP.mult)
                for kc in range(qt + 1):
                    nc.tensor.transpose(
                        pst[kc][:, qt * P:qt * P + P],
                        pr[:, kc * P:kc * P + P], ident[:])
            prT = []
            for kc in range(NQ):
                t = pool.tile([P, S], f32r, name="t", tag=f"prT{kc}", bufs=1)
                nc.vector.tensor_copy(t[:, kc * P:], pst[kc][:, kc * P:])
                prT.append(t)
            pav = psAv.tile([P, S], f32, name="t", tag="av")
            for kc in range(NQ):
                nc.tensor.matmul(
                    pav[:, kc * P:], V[kc][:, h * DK:(h + 1) * DK],
                    prT[kc][:, kc * P:],
                    start=(kc == 0), stop=(kc == NQ - 1))
            nc.vector.tensor_copy(att_dst, pav[:])

        def layer(l, bmask, apply_pos, xsrc_dram, vals_src, out_dram,
                  final=False):
            """xsrc_dram: [D, TOK] DRAM source for the query/key input.
            vals_src: 'self' or a DRAM tile to stream per b.
            out_dram: DRAM target AP base for the layer output."""
            wdam = dam_prep(l)
            pdam = tc.alloc_tile_pool(name=f"dam{l}", bufs=1)
            damGs = []
            for h in range(H):
                g = pdam.tile([P, 2 * S - 1], u8, name="t", tag=f"damG{h}")
                nc.gpsimd.indirect_dma_start(
                    out=g[:], out_offset=None, in_=wdam[:],
                    in_offset=bass.IndirectOffsetOnAxis(
                        ap=idxt[h][:, :1], axis=1))
                damGs.append(g)
            for b in range(NB):
                bs = b * S
                pool = tc.alloc_tile_pool(name=f"att{l}{b}", bufs=2)
                xq_tiles = []
                for idt in range(ND):
                    t = pool.tile([P, S], f32r, name="t", tag=f"xa{idt}",
                                  bufs=1)
                    wdma(t[:],
                         xsrc_dram[idt * P:(idt + 1) * P, bs:bs + S])
                    xq_tiles.append(t)
                # ---- K projection (q==k), kwt streamed in od-halves
                K = []
                for half in range(2):
                    wk = []
                    for idt in range(ND):
                        t = pool.tile([P, S], f32r, name="t", tag=f"wbig{idt}",
                                          bufs=2)
                        wdma(
                            t[:],
                            kwt_e[l, idt * P:(idt + 1) * P,
                                      half * S:(half + 1) * S])
                        wk.append(t)
                    for oc in range(4):
                        od = half * 4 + oc
                        ps = psQ.tile([P, S], f32, name="t", tag="qk")
                        mm_group(ps[:], [
                            (wk[idt][:, oc * P:(oc + 1) * P],
                             xq_tiles[idt][:]) for idt in range(ND)])
                        kt = pool.tile([P, S], f32r, name="t", tag=f"K{od}",
                                       bufs=1)
                        nc.vector.tensor_copy(kt[:], ps[:])
                        K.append(kt)
                # ---- VALS for v-projection
                if vals_src == "self":
                    vals = [xq_tiles[idt][:] for idt in range(ND)]
                else:
                    vt = []
                    for idt in range(ND):
                        t = pool.tile([P, S], f32r, name="t", tag=f"att{idt}", bufs=1)
                        wdma(
                            t[:],
                            vals_src[idt * P:(idt + 1) * P, bs:bs + S])
                        vt.append(t)
                    vals = [t[:] for t in vt]
                # ---- V projection (token-major), vwt streamed in d-halves
                V = [pool.tile([P, D], f32r, name="t", tag=f"V{st}", bufs=1)
                     for st in range(NQ)]
                for half in range(2):
                    wv = []
                    for idt in range(ND):
                        t = pool.tile([P, S], f32r, name="t", tag=f"wbig{idt}",
                                          bufs=2)
                        wdma(
                            t[:],
                            vwt_e[l, idt * P:(idt + 1) * P,
                                      half * S:(half + 1) * S])
                        wv.append(t)
                    for st in range(NQ):
                        ps = psQ.tile([P, S], f32, name="t", tag="qk")
                        mm_group(ps[:], [
                            (vals[idt][:, st * P:(st + 1) * P], wv[idt][:])
                            for idt in range(ND)])
                        nc.vector.tensor_copy(
                            V[st][:, half * S:(half + 1) * S], ps[:])
                # ---- attention heads
                att = [pool.tile([P, S], f32r, name="t", tag=f"att{od}", bufs=1)
                       for od in range(ND)]
                for h in range(H):
                    attention_head(pool, l, bmask, h, K, V, att[h][:], damGs[h])
                # ---- o-projection + residual, owt streamed in od-halves
                r_t = []
                for half in range(2):
                    wo = []
                    for idt in range(ND):
                        t = pool.tile([P, S], f32r, name="t", tag=f"wbig{idt}",
                                          bufs=2)
                        wdma(
                            t[:],
                            owt_e[l, idt * P:(idt + 1) * P,
                                      half * S:(half + 1) * S])
                        wo.append(t)
                    for oc in range(4):
                        od = half * 4 + oc
                        ps = psQ.tile([P, S], f32, name="t", tag="qk")
                        mm_group(ps[:], [
                            (wo[idt][:, oc * P:(oc + 1) * P], att[idt][:])
                            for idt in range(ND)])
                        rt = pool.tile([P, S], f32r, name="t",
                                       tag=f"wbig{od}", bufs=2)
                        nc.vector.tensor_tensor(
                            rt[:], xq_tiles[od][:], ps[:], OP.add)
                        r_t.append(rt)
                # ---- LN1
                if apply_pos:
                    xp = [pg.tile([P, S], f32r, name="t", tag=f"xp{od}")
                          for od in range(ND)]
                    layernorm(pool, r_t, [t[:] for t in xp])
                else:
                    ot = [pool.tile([P, S], f32 if final else f32r, name="t",
                                    tag="outt", bufs=2)
                          for _ in range(ND)]
                    layernorm(pool, r_t, [t[:] for t in ot])
                    for od in range(ND):
                        nc.sync.dma_start(
                            out=out_dram[od * P:(od + 1) * P, bs:bs + S],
                            in_=ot[od][:])
                pool.release()

                if not apply_pos:
                    continue
                # ---- FFN + LN2
                fp = tc.alloc_tile_pool(name=f"ffn{l}{b}", bufs=2)
                xpb = []
                for od in range(ND):
                    t = fp.tile([P, S], bf16, name="t", tag=f"xpb{od}", bufs=1)
                    nc.vector.tensor_copy(t[:], xp[od][:])
                    xpb.append(t)
                h1 = []
                for fc in range(8):
                    w1c = []
                    for idt in range(ND):
                        t = fp.tile([P, S], bf16, name="t", tag=f"w1c{idt}")
                        wdma(
                            t[:],
                            w1t_e[l, idt * P:(idt + 1) * P,
                                      fc * S:(fc + 1) * S])
                        w1c.append(t)
                    for fl in range(4):
                        ps = psQ.tile([P, S], f32, name="t", tag="qk")
                        mm_group(ps[:], [
                            (w1c[idt][:, fl * P:(fl + 1) * P], xpb[idt][:])
                            for idt in range(ND)])
                        ht = fp.tile([P, S], bf16, name="t",
                                     tag=f"h1_{fc * 4 + fl}", bufs=1)
                        nc.vector.tensor_scalar(ht[:], ps[:], 0.0, None,
                                                OP.max)
                        h1.append(ht)
                r_t = []
                for og in range(2):
                    pso = [psT.tile([P, S], f32, name="t", tag=f"pt{oc}")
                           for oc in range(4)]
                    for fc in range(8):
                        w2c = []
                        for fl in range(4):
                            ft = fc * 4 + fl
                            t = fp.tile([P, S], bf16, name="t", tag=f"w2c{fl}")
                            wdma(
                                t[:],
                                w2t_e[l, ft * P:(ft + 1) * P,
                                          og * S:(og + 1) * S])
                            w2c.append(t)
                        for fl in range(4):
                            ft = fc * 4 + fl
                            for oc in range(4):
                                nc.tensor.matmul(
                                    pso[oc][:],
                                    w2c[fl][:, oc * P:(oc + 1) * P],
                                    h1[ft][:],
                                    start=(fc == 0 and fl == 0),
                                    stop=(fc == 7 and fl == 3))
                    for oc in range(4):
                        od = og * 4 + oc
                        rt = fp.tile([P, S], f32r, name="t", tag=f"r{od}",
                                     bufs=1)
                        nc.vector.tensor_tensor(
                            rt[:], xp[od][:], pso[oc][:], OP.add)
                        r_t.append(rt)
                ot = [fp.tile([P, S], f32 if final else f32r, name="t",
                              tag="outt", bufs=4)
                      for _ in range(ND)]
                layernorm(fp, r_t, [t[:] for t in ot])
                for od in range(ND):
                    nc.sync.dma_start(
                        out=out_dram[od * P:(od + 1) * P, bs:bs + S],
                        in_=ot[od][:])
                fp.release()
            pdam.release()

        # ================= driver =================
        for _rep in range(repeat):
            layer(0, 1, True, xqa_e, "self", y_dram)
            if nlayers >= 2:
                layer(1, 1, False, xq_e, "self", x1_dram)
            if nlayers >= 3:
                layer(2, 0, True, x1_dram, y_dram, out_e, final=True)
            if nlayers == 1:
                nc.gpsimd.dma_start(out=out_e[:], in_=y_dram[:])
            elif nlayers == 2:
                nc.gpsimd.dma_start(out=out_e[:], in_=x1_dram[:])

        psAv.release()
        psT.release()
        psQ.release()
        pdram.release()
        pg.release()

    nc.finalize()
    return nc, tap_outs


def _get_nc(nlayers=3, taps=(), repeat=1):
    key = (nlayers, tuple(sorted(taps)), repeat)
    if key not in _CACHE:
        _CACHE[key] = _build(nlayers, taps, repeat)
    return _CACHE[key]


def _make_in_maps(inputs):
    qa = np.asarray(inputs["qa_embed_data"])
    qd = np.asarray(inputs["q_embed_data"])
    al = np.asarray(inputs["alphas"])
    ge = np.asarray(inputs["gumbel_E"])
    a0f = al[..., 0]; a1f = al[..., 1]
    e0f = ge[..., 0]; e1f = ge[..., 1]
    i_ = np.arange(S)
    shared = {
        "kwt": np.asarray(inputs["kW"]).transpose(0, 2, 1),
        "vwt": np.asarray(inputs["vW"]).transpose(0, 2, 1),
        "owt": np.asarray(inputs["oW"]).transpose(0, 2, 1),
        "w1t": np.asarray(inputs["w1"]).transpose(0, 2, 1),
        "w2t": np.asarray(inputs["w2"]).transpose(0, 2, 1),
        "a0f": a0f, "a1f": a1f, "e0f": e0f, "e1f": e1f,
        "a0r": a0f[:, :, ::-1], "a1r": a1f[:, :, ::-1],
        "e0r": e0f[:, :, ::-1], "e1r": e1f[:, :, ::-1],
        "gam": np.asarray(inputs["gammas"]).reshape(1, LN_ * H),
        "posn": -np.abs(i_[:, None] - i_[None, :]),
    }
    import ml_dtypes
    casts = {"w1t": ml_dtypes.bfloat16, "w2t": ml_dtypes.bfloat16,
             "posn": np.float16}
    shared = {k: np.ascontiguousarray(v, dtype=casts.get(k, np.float32))
              for k, v in shared.items()}

    def feat_major(x, c):
        pair = np.asarray(x[NB * c:NB * c + NB])        # [2, S, D]
        return np.ascontiguousarray(
            pair.transpose(2, 0, 1).reshape(D, TOK), dtype=np.float32)

    in_maps = []
    for c in range(8):
        m = dict(shared)
        m["xqa"] = feat_major(qa, c)
        m["xq"] = feat_major(qd, c)
        in_maps.append(m)
    return in_maps


def _gather_out(results):
    outs = []
    for r in results:
        o = r["out"].reshape(D, NB, S).transpose(1, 2, 0)
        outs.append(o)
    return np.ascontiguousarray(np.concatenate(outs, axis=0))


def kernel(**inputs):
    from concourse.bass_utils import run_bass_kernel_spmd
    nc, _ = _get_nc()
    in_maps = _make_in_maps(inputs)
    res = run_bass_kernel_spmd(nc, in_maps, core_ids=list(range(8)))
    return _gather_out(res.results)



# revision 25
# speedup vs baseline: 1.3334x; 1.1137x over previous
"""Trainium2 Bass kernel for nn_Architecture_50629074485965 (3-layer AKT-style
transformer, B=16 S=512 D=1024 H=8 DFF=4096).

Sharding: data-parallel over batch — 2 batches per core, 8 cores, no
collectives.  Activations are feature-major [D on partitions, tokens free] so
every matmul chains without activation transposes (weights host-pre-
transposed).  Score path (K, q@k) runs in float32r; the value path (V, att,
probs, FFN) runs bf16.  Layer outputs bounce through DRAM.

All tile pools are persistent: tags rotate across batches and layers instead
of pool release/realloc, so the scheduler can overlap batch b1's projections
and attention with batch b0's FFN (PE-heavy vs ACT/DVE-heavy phases).

Every ACT transcendental is Exp or Ln (sqrt(x) = exp(0.5 ln x)) so a single
activation table set serves the whole kernel (no ~2.7us table swaps).

Attention per (b,h), per 128-row q-tile (q-major [q, k] layout):
  psum  = q @ k^T                         (PE f32r)
  e1    = Exp(psum/sqrt(dk))              (ACT, full width)
  e1c   = causal(e1)                      (GPSIMD affine_select, width w)
  r1    = sum_j e1*dam01                  (DVE stt accum -> throwaway edam;
                                           dam01 = u8 row-window gather from a
                                           per-head Toeplitz vector)
  cum   = cumsum(e1c)                     (DVE tensor_tensor_scan)
  d2    = (cum - rowtot) * (-|i-j|) >= 0  (DVE stt, posn = -|i-j| in f16)
  te    = exp(-exp(0.5 ln d2 + lgam - 0.5 ln r1))   (ACT Ln/Exp/Exp)
  t2u   = max(te,1e-5) * psum             (DVE stt; diag block causal-masked
                                           in place by GPSIMD)
  e2,r2 = Exp(t2u/sqrt(dk)) + row-sum     (ACT accum_out)
  probs = e2 * (1/max(r2,1e-30)) -> bf16  (DVE)
  probsT blocks: PE transpose -> psum -> sbuf
  att   = v-chunks(lhsT) @ probsT -> feature-major  (PE, bf16)
"""
import sys
sys.path.insert(0, "/opt/trn_rl_repo")
import numpy as np

B, S, D, H, DFF, LN_ = 16, 512, 1024, 8, 4096, 3
DK = D // H
NB = 2
TOK = NB * S
P = 128
ND = D // P      # 8
NQ = S // P      # 4
ISD = 1.0 / float(np.sqrt(DK))
WPAD = 2048

_CACHE = {}


def _build(nlayers=3, taps=(), repeat=1):
    import concourse.bass as bass
    import concourse.mybir as mybir
    from concourse import bacc
    from concourse.tile import TileContext

    dt = mybir.dt
    f32, f32r, bf16, f16, u8, i32 = (dt.float32, dt.float32r, dt.bfloat16,
                                     dt.float16, dt.uint8, dt.int32)
    AF = mybir.ActivationFunctionType
    OP = mybir.AluOpType

    nc = bacc.Bacc(None, target_bir_lowering=False)

    # Every transcendental in this kernel is Exp or Ln. The act-table-load
    # pass picks the first act_info set containing each function, which makes
    # Exp/Ln alternation swap tables every few ops (~2.7us per swap on HW).
    # Steer both to the combined natural_log_exp set by hiding them from the
    # single-function sets (dict identity is the functools.cache singleton;
    # set indices — what walrus consumes — are unchanged).
    from concourse.hw_specs import get_activation_tables
    _tabs = get_activation_tables(nc.m.arch)
    for _name, _fns in _tabs.items():
        if _name != "natural_log_exp_and_others":
            _fns.discard(AF.Exp)
            _fns.discard(AF.Ln)

    def par(name, shape, out=False, dtype=None):
        return nc.declare_dram_parameter(name, list(shape), dtype or f32,
                                         isOutput=out)

    xqa_e = par("xqa", [D, TOK], dtype=f32r)
    xq_e = par("xq", [D, TOK], dtype=f32r)
    kwt_e = par("kwt", [LN_, D, D], dtype=f32r)
    vwt_e = par("vwt", [LN_, D, D], dtype=f32r)
    owt_e = par("owt", [LN_, D, D], dtype=bf16)
    w1t_e = par("w1t", [LN_, D, DFF], dtype=bf16)
    w2t_e = par("w2t", [LN_, DFF, D], dtype=bf16)
    a0f_e = par("a0f", [LN_, H, S]); a1f_e = par("a1f", [LN_, H, S])
    e0f_e = par("e0f", [LN_, H, S]); e1f_e = par("e1f", [LN_, H, S])
    a0r_e = par("a0r", [LN_, H, S]); a1r_e = par("a1r", [LN_, H, S])
    e0r_e = par("e0r", [LN_, H, S]); e1r_e = par("e1r", [LN_, H, S])
    gam_e = par("gam", [1, LN_ * H])
    posn_e = par("posn", [S, S], dtype=f16)
    out_e = par("out", [D, TOK], out=True)
    tap_outs = {}

    with TileContext(nc) as tc:
        pg = tc.alloc_tile_pool(name="glob", bufs=1)
        pdram = tc.alloc_tile_pool(name="dram", bufs=1, space="DRAM")
        psQ = tc.alloc_tile_pool(name="psQ", bufs=3, space="PSUM")
        psT = tc.alloc_tile_pool(name="psT", bufs=1, space="PSUM")
        psAv = tc.alloc_tile_pool(name="psAv", bufs=1, space="PSUM")
        pool = tc.alloc_tile_pool(name="main", bufs=2)

        _dmaq = [nc.sync, nc.scalar]
        _dmac = [0]

        def wdma(out, in_):
            eng = _dmaq[_dmac[0] % len(_dmaq)]
            _dmac[0] += 1
            eng.dma_start(out=out, in_=in_)

        def mm_group(psum_ap, pairs):
            n = len(pairs)
            for i, (lt, rh) in enumerate(pairs):
                nc.tensor.matmul(psum_ap, lt, rh,
                                 start=(i == 0), stop=(i == n - 1))

        # ---------------- constants (global pool) ----------------
        ident_f = pg.tile([P, P], f32, name="t", tag="identf")
        nc.gpsimd.memset(ident_f[:], 0.0)
        nc.gpsimd.affine_select(
            out=ident_f[:], in_=ident_f[:], compare_op=OP.not_equal,
            fill=1.0, base=0, channel_multiplier=1, pattern=[[-1, P]])
        ident_bf = pg.tile([P, P], bf16, name="t", tag="identbf")
        nc.vector.tensor_copy(ident_bf[:], ident_f[:])

        ones_f = pg.tile([P, 1], f32, name="t", tag="onesf")
        nc.gpsimd.memset(ones_f[:], 1.0)
        ones_col = pg.tile([P, 1], f32r, name="t", tag="ones")
        nc.vector.tensor_copy(ones_col[:], ones_f[:])
        eps5 = pg.tile([P, 1], f32, name="t", tag="eps5")
        nc.gpsimd.memset(eps5[:], 1e-5)

        posn = []
        for qt in range(NQ):
            t = pg.tile([P, S], f16, name="t", tag=f"posn{qt}")
            nc.sync.dma_start(out=t[:], in_=posn_e[qt * P:(qt + 1) * P, :])
            posn.append(t)

        idxt = []
        for h in range(H):
            t = pg.tile([P, 1], i32, name="t", tag=f"idx{h}")
            nc.gpsimd.iota(t[:], pattern=[[1, 1]],
                           base=h * WPAD + (S - 1) - P * (NQ - 1),
                           channel_multiplier=-1)
            idxt.append(t)

        grow = pg.tile([1, LN_ * H], f32, name="t", tag="grow")
        nc.sync.dma_start(out=grow[:], in_=gam_e[:])
        one_c = pg.tile([P, 1], f32, name="t", tag="one_c")
        nc.gpsimd.memset(one_c[:], 1.0)
        # softplus(x) = ln(1 + exp(x)) computed manually (no Softplus table)
        gsp = pg.tile([1, LN_ * H], f32, name="t", tag="gsp")
        nc.scalar.activation(gsp[:], grow[:], AF.Exp)
        nc.scalar.activation(gsp[:], gsp[:], AF.Ln, bias=one_c[:1, :])
        # lgam = ln(softplus(gamma)); te = exp(-exp(0.5*ln(d2)-0.5*ln(r1)+lgam))
        lgam = pg.tile([1, LN_ * H], f32, name="t", tag="lgam")
        nc.scalar.activation(lgam[:], gsp[:], AF.Ln)
        lgam_bc = []
        for i in range(LN_ * H):
            t = pg.tile([P, 1], f32, name="t", tag=f"gbc{i}")
            nc.gpsimd.partition_broadcast(t[:], lgam[0:1, i:i + 1])
            lgam_bc.append(t)

        y_dram = pdram.tile([D, TOK], f32r, name="t", tag="ydram")
        x1_dram = pdram.tile([D, TOK], f32r, name="t", tag="x1dram")

        # ---------------- helpers ----------------
        def dam_prep(l):
            wdam = pdram.tile([1, H * WPAD], u8, name="t", tag="wdam",
                              bufs=2)

            def half(a0e, a1e, e0e, e1e):
                tA = pool.tile([H, S], f32, name="t", tag="tmpA", bufs=4)
                tB = pool.tile([H, S], f32, name="t", tag="tmpB", bufs=3)
                tC = pool.tile([H, S], f32, name="t", tag="tmpA", bufs=4)
                tD = pool.tile([H, S], f32, name="t", tag="tmpB", bufs=3)
                nc.sync.dma_start(out=tA[:], in_=e0e[l])
                nc.sync.dma_start(out=tB[:], in_=e1e[l])
                nc.scalar.activation(tA[:], tA[:], AF.Ln, bias=eps5[:H, :])
                nc.scalar.activation(tB[:], tB[:], AF.Ln, bias=eps5[:H, :])
                nc.vector.tensor_tensor(tA[:], tA[:], tB[:], OP.subtract)
                nc.sync.dma_start(out=tC[:], in_=a1e[l])
                nc.sync.dma_start(out=tD[:], in_=a0e[l])
                nc.vector.tensor_tensor(tC[:], tC[:], tD[:], OP.subtract)
                nc.vector.tensor_tensor(tA[:], tA[:], tC[:], OP.add)
                c = pool.tile([H, S], u8, name="t", tag="edam", bufs=2)
                nc.vector.tensor_scalar(c[:], tA[:], 0.0, None, OP.is_gt)
                return c

            cf = half(a0f_e, a1f_e, e0f_e, e1f_e)
            cr = half(a0r_e, a1r_e, e0r_e, e1r_e)
            dst_r = bass.AP(tensor=wdam.tensor, offset=0,
                            ap=[[WPAD, H], [1, S - 1]])
            dst_f = bass.AP(tensor=wdam.tensor, offset=S - 1,
                            ap=[[WPAD, H], [1, S]])
            nc.sync.dma_start(out=dst_r, in_=cr[:, 0:S - 1])
            nc.sync.dma_start(out=dst_f, in_=cf[:])
            return wdam

        def layernorm(r_t, dsts):
            """r_t: 8 [P,S] f32r tiles; writes (x-mu)/sigma into dsts APs."""
            s1 = psT.tile([1, S], f32, name="t", tag="pt0")
            mm_group(s1[:], [(ones_col[:], r_t[od][:]) for od in range(ND)])
            s2 = psT.tile([1, S], f32, name="t", tag="pt1")
            for od in range(ND):
                sq = pool.tile([P, S], f32r, name="t", tag="tmpA", bufs=4)
                nc.vector.tensor_tensor(sq[:], r_t[od][:], r_t[od][:],
                                        OP.mult)
                nc.tensor.matmul(s2[:], ones_col[:], sq[:],
                                 start=(od == 0), stop=(od == ND - 1))
            mean = pool.tile([1, S], f32, name="t", tag="lnr0", bufs=2)
            nc.vector.tensor_scalar(mean[:], s1[:], 1.0 / D, None, OP.mult)
            msq = pool.tile([1, S], f32, name="t", tag="lnr1", bufs=2)
            nc.vector.tensor_scalar(msq[:], s2[:], 1.0 / D, None, OP.mult)
            m2 = pool.tile([1, S], f32, name="t", tag="lnr2", bufs=2)
            nc.vector.tensor_tensor(m2[:], mean[:], mean[:], OP.mult)
            nc.vector.tensor_tensor(msq[:], msq[:], m2[:], OP.subtract)
            # rstd = exp(-0.5*ln(var+eps)) — stays in the ln/exp table set
            nc.scalar.activation(msq[:], msq[:], AF.Ln, bias=eps5[:1, :])
            nc.scalar.activation(m2[:], msq[:], AF.Exp, scale=-0.5)
            nc.vector.tensor_scalar(mean[:], mean[:], -1.0, None, OP.mult)
            nc.vector.tensor_tensor(mean[:], mean[:], m2[:], OP.mult)
            Ab = pool.tile([P, S], f32, name="t", tag="Ab", bufs=1)
            nc.gpsimd.partition_broadcast(Ab[:], m2[:])
            Cb = pool.tile([P, S], f32, name="t", tag="Cb", bufs=1)
            nc.gpsimd.partition_broadcast(Cb[:], mean[:])
            for od in range(ND):
                t1 = pool.tile([P, S], f32, name="t", tag="lnt", bufs=1)
                nc.vector.tensor_tensor(t1[:], r_t[od][:], Ab[:], OP.mult)
                nc.gpsimd.tensor_tensor(dsts[od], t1[:], Cb[:], OP.add)

        def attention_head(l, bmask, h, K, V, att_dst, damG):
            pst = [psT.tile([P, S], bf16, name="t", tag=f"pt{kc}")
                   for kc in range(NQ)]
            ktile = K[h]
            for qt in range(NQ):
                w = P * (qt + 1)
                ps = psQ.tile([P, S], f32, name="t", tag="qk")
                mm_group(ps[:], [(ktile[:, qt * P:qt * P + P], ktile[:])])
                doff = P * (NQ - 1) - P * qt
                e1 = pool.tile([P, S], f32, name="t", tag="e1", bufs=2)
                nc.scalar.activation(e1[:], ps[:], AF.Exp, scale=ISD)
                e1c = pool.tile([P, S], f32, name="t", tag="tmpA", bufs=4)
                nc.gpsimd.affine_select(
                    out=e1c[:, :w], in_=e1[:, :w], compare_op=OP.is_gt,
                    fill=0.0, base=qt * P + bmask, channel_multiplier=1,
                    pattern=[[-1, w]])
                r1 = pool.tile([P, 1], f32, name="t", tag="sm_r1")
                edam = pool.tile([P, S], bf16, name="t", tag="edam", bufs=2)
                nc.vector.scalar_tensor_tensor(
                    edam[:], e1[:], 1.0, damG[:, doff:doff + S],
                    OP.mult, OP.mult, accum_out=r1[:])
                cum = pool.tile([P, S], f32, name="t", tag="tmpB", bufs=3)
                nc.vector.tensor_tensor_scan(
                    cum[:, :w], e1c[:, :w], e1c[:, :w], 0.0, OP.add, OP.bypass)
                lnr1 = pool.tile([P, 1], f32, name="t", tag="sm_rc1")
                nc.scalar.activation(lnr1[:], r1[:], AF.Ln)
                brow = pool.tile([P, 1], f32, name="t", tag="sm_brow")
                nc.vector.scalar_tensor_tensor(
                    brow[:], lnr1[:], -0.5, lgam_bc[l * H + h][:],
                    OP.mult, OP.add)
                d2 = pool.tile([P, S], f32, name="t", tag="tmpA", bufs=4)
                nc.vector.scalar_tensor_tensor(
                    d2[:, :w], cum[:, :w], cum[:, w - 1:w], posn[qt][:, :w],
                    OP.subtract, OP.mult)
                dist = pool.tile([P, S], f32, name="t", tag="tmpB", bufs=3)
                nc.scalar.activation(dist[:, :w], d2[:, :w], AF.Ln)
                sga = pool.tile([P, S], f32, name="t", tag="tmpA", bufs=4)
                nc.scalar.activation(sga[:, :w], dist[:, :w], AF.Exp,
                                     scale=0.5, bias=brow[:])
                te = pool.tile([P, S], f32, name="t", tag="tmpB", bufs=3)
                nc.scalar.activation(te[:, :w], sga[:, :w], AF.Exp,
                                     scale=-1.0)
                t2u = pool.tile([P, S], f32, name="t", tag="tmpA", bufs=4)
                nc.vector.scalar_tensor_tensor(
                    t2u[:, :w], te[:, :w], 1e-5, ps[:, :w], OP.max, OP.mult)
                # causal boundary only cuts the 128-wide diagonal block;
                # mask it in place instead of re-writing the full width.
                nc.gpsimd.affine_select(
                    out=t2u[:, w - P:w], in_=t2u[:, w - P:w],
                    compare_op=OP.is_gt, fill=-1e30, base=bmask,
                    channel_multiplier=1, pattern=[[-1, P]])
                e2 = pool.tile([P, S], f32, name="t", tag="tmpB", bufs=3)
                r2 = pool.tile([P, 1], f32, name="t", tag="sm_r2")
                nc.scalar.activation(e2[:, :w], t2u[:, :w], AF.Exp,
                                     scale=ISD, accum_out=r2[:])
                nc.vector.tensor_scalar(r2[:], r2[:], 1e-30, None, OP.max)
                rec2 = pool.tile([P, 1], f32, name="t", tag="sm_rc2")
                nc.vector.reciprocal(rec2[:], r2[:])
                pr = pool.tile([P, S], bf16, name="t", tag="probs", bufs=2)
                nc.vector.tensor_scalar(pr[:, :w], e2[:, :w], rec2[:],
                                        None, OP.mult)
                for kc in range(qt + 1):
                    nc.tensor.transpose(
                        pst[kc][:, qt * P:qt * P + P],
                        pr[:, kc * P:kc * P + P], ident_bf[:])
            prT = []
            for kc in range(NQ):
                t = pool.tile([P, S], bf16, name="t", tag=f"prT{kc}", bufs=1)
                nc.vector.tensor_copy(t[:, kc * P:], pst[kc][:, kc * P:])
                prT.append(t)
            pav = psAv.tile([P, S], f32, name="t", tag="av")
            for kc in range(NQ):
                nc.tensor.matmul(
                    pav[:, kc * P:], V[kc][:, h * DK:(h + 1) * DK],
                    prT[kc][:, kc * P:],
                    start=(kc == 0), stop=(kc == NQ - 1))
            nc.vector.tensor_copy(att_dst, pav[:])

        def layer(l, bmask, apply_pos, xsrc_dram, vals_src, out_dram,
                  final=False):
            """xsrc_dram: [D, TOK] DRAM source for the query/key input.
            vals_src: 'self' or a DRAM tile to stream per b.
            out_dram: DRAM target AP base for the layer output."""
            wdam = dam_prep(l)
            damGs = []
            for h in range(H):
                g = pool.tile([P, 2 * S - 1], u8, name="t", tag=f"damG{h}",
                              bufs=1)
                nc.gpsimd.indirect_dma_start(
                    out=g[:], out_offset=None, in_=wdam[:],
                    in_offset=bass.IndirectOffsetOnAxis(
                        ap=idxt[h][:, :1], axis=1))
                damGs.append(g)
            for b in range(NB):
                bs = b * S
                xq_tiles = []
                for idt in range(ND):
                    t = pool.tile([P, S], f32r, name="t", tag=f"xa{idt}",
                                  bufs=1)
                    wdma(t[:],
                         xsrc_dram[idt * P:(idt + 1) * P, bs:bs + S])
                    xq_tiles.append(t)
                # ---- K projection (q==k), kwt streamed in od-halves
                K = []
                for half in range(2):
                    wk = []
                    for idt in range(ND):
                        t = pool.tile([P, S], f32r, name="t",
                                      tag=f"kw{idt}", bufs=1)
                        wdma(
                            t[:],
                            kwt_e[l, idt * P:(idt + 1) * P,
                                      half * S:(half + 1) * S])
                        wk.append(t)
                    for oc in range(4):
                        od = half * 4 + oc
                        ps = psQ.tile([P, S], f32, name="t", tag="qk")
                        mm_group(ps[:], [
                            (wk[idt][:, oc * P:(oc + 1) * P],
                             xq_tiles[idt][:]) for idt in range(ND)])
                        kt = pool.tile([P, S], f32r, name="t", tag=f"K{od}",
                                       bufs=1)
                        nc.vector.tensor_copy(kt[:], ps[:])
                        K.append(kt)
                # ---- VALS for v-projection
                if vals_src == "self":
                    vals = [xq_tiles[idt][:] for idt in range(ND)]
                else:
                    vt = []
                    for idt in range(ND):
                        t = pool.tile([P, S], f32r, name="t", tag=f"r{idt}",
                                      bufs=1)
                        wdma(
                            t[:],
                            vals_src[idt * P:(idt + 1) * P, bs:bs + S])
                        vt.append(t)
                    vals = [t[:] for t in vt]
                # ---- V projection (token-major), vwt streamed in d-halves
                V = [pool.tile([P, D], bf16, name="t", tag=f"V{st}", bufs=1)
                     for st in range(NQ)]
                for half in range(2):
                    wv = []
                    for idt in range(ND):
                        t = pool.tile([P, S], f32r, name="t",
                                      tag=f"kw{idt}", bufs=1)
                        wdma(
                            t[:],
                            vwt_e[l, idt * P:(idt + 1) * P,
                                      half * S:(half + 1) * S])
                        wv.append(t)
                    for st in range(NQ):
                        ps = psQ.tile([P, S], f32, name="t", tag="qk")
                        mm_group(ps[:], [
                            (vals[idt][:, st * P:(st + 1) * P], wv[idt][:])
                            for idt in range(ND)])
                        nc.vector.tensor_copy(
                            V[st][:, half * S:(half + 1) * S], ps[:])
                # ---- attention heads
                att = [pool.tile([P, S], bf16, name="t", tag=f"att{od}",
                                 bufs=4)
                       for od in range(ND)]
                for h in range(H):
                    attention_head(l, bmask, h, K, V, att[h][:], damGs[h])
                # ---- o-projection + residual, owt streamed in od-halves
                r_t = []
                for half in range(2):
                    wo = []
                    for idt in range(ND):
                        t = pool.tile([P, S], bf16, name="t",
                                      tag=f"wbig{idt}", bufs=2)
                        wdma(
                            t[:],
                            owt_e[l, idt * P:(idt + 1) * P,
                                      half * S:(half + 1) * S])
                        wo.append(t)
                    for oc in range(4):
                        od = half * 4 + oc
                        ps = psQ.tile([P, S], f32, name="t", tag="qk")
                        mm_group(ps[:], [
                            (wo[idt][:, oc * P:(oc + 1) * P], att[idt][:])
                            for idt in range(ND)])
                        rt = pool.tile([P, S], f32r, name="t",
                                       tag=f"r{od}", bufs=1)
                        nc.vector.tensor_tensor(
                            rt[:], xq_tiles[od][:], ps[:], OP.add)
                        r_t.append(rt)
                # ---- LN1
                if apply_pos:
                    xp = [pg.tile([P, S], f32r, name="t", tag=f"xp{od}")
                          for od in range(ND)]
                    layernorm(r_t, [t[:] for t in xp])
                else:
                    ot = [pool.tile([P, S], f32 if final else f32r, name="t",
                                    tag="outt", bufs=2)
                          for _ in range(ND)]
                    layernorm(r_t, [t[:] for t in ot])
                    for od in range(ND):
                        nc.sync.dma_start(
                            out=out_dram[od * P:(od + 1) * P, bs:bs + S],
                            in_=ot[od][:])
                    continue

                # ---- FFN + LN2 (bf16 weights and activations)
                xpb = []
                for od in range(ND):
                    t = pool.tile([P, S], bf16, name="t", tag=f"xpb{od}",
                                  bufs=1)
                    nc.vector.tensor_copy(t[:], xp[od][:])
                    xpb.append(t)
                h1 = []
                for fc in range(8):
                    w1c = []
                    for idt in range(ND):
                        t = pool.tile([P, S], bf16, name="t",
                                      tag=f"wbig{idt}", bufs=2)
                        wdma(
                            t[:],
                            w1t_e[l, idt * P:(idt + 1) * P,
                                      fc * S:(fc + 1) * S])
                        w1c.append(t)
                    for fl in range(4):
                        ft = fc * 4 + fl
                        ps = psQ.tile([P, S], f32, name="t", tag="qk")
                        mm_group(ps[:], [
                            (w1c[idt][:, fl * P:(fl + 1) * P], xpb[idt][:])
                            for idt in range(ND)])
                        ht = pool.tile([P, S], bf16, name="t",
                                       tag=f"att{ft % 8}", bufs=4)
                        nc.vector.tensor_scalar(ht[:], ps[:], 0.0, None,
                                                OP.max)
                        h1.append(ht)
                r_t = []
                for og in range(2):
                    pso = [psT.tile([P, S], f32, name="t", tag=f"pt{oc}")
                           for oc in range(4)]
                    for fc in range(8):
                        w2c = []
                        for fl in range(4):
                            ft = fc * 4 + fl
                            t = pool.tile([P, S], bf16, name="t",
                                          tag=f"wbig{4 + fl}", bufs=2)
                            wdma(
                                t[:],
                                w2t_e[l, ft * P:(ft + 1) * P,
                                          og * S:(og + 1) * S])
                            w2c.append(t)
                        for fl in range(4):
                            ft = fc * 4 + fl
                            for oc in range(4):
                                nc.tensor.matmul(
                                    pso[oc][:],
                                    w2c[fl][:, oc * P:(oc + 1) * P],
                                    h1[ft][:],
                                    start=(fc == 0 and fl == 0),
                                    stop=(fc == 7 and fl == 3))
                    for oc in range(4):
                        od = og * 4 + oc
                        rt = pool.tile([P, S], f32r, name="t", tag=f"r{od}",
                                       bufs=1)
                        nc.vector.tensor_tensor(
                            rt[:], xp[od][:], pso[oc][:], OP.add)
                        r_t.append(rt)
                ot = [pool.tile([P, S], f32 if final else f32r, name="t",
                                tag="outt", bufs=2)
                      for _ in range(ND)]
                layernorm(r_t, [t[:] for t in ot])
                for od in range(ND):
                    nc.sync.dma_start(
                        out=out_dram[od * P:(od + 1) * P, bs:bs + S],
                        in_=ot[od][:])

        # ================= driver =================
        for _rep in range(repeat):
            layer(0, 1, True, xqa_e, "self", y_dram)
            if nlayers >= 2:
                layer(1, 1, False, xq_e, "self", x1_dram)
            if nlayers >= 3:
                layer(2, 0, True, x1_dram, y_dram, out_e, final=True)
            if nlayers == 1:
                nc.gpsimd.dma_start(out=out_e[:], in_=y_dram[:])
            elif nlayers == 2:
                nc.gpsimd.dma_start(out=out_e[:], in_=x1_dram[:])

        pool.release()
        psAv.release()
        psT.release()
        psQ.release()
        pdram.release()
        pg.release()

    nc.finalize()
    return nc, tap_outs


def _get_nc(nlayers=3, taps=(), repeat=1):
    key = (nlayers, tuple(sorted(taps)), repeat)
    if key not in _CACHE:
        _CACHE[key] = _build(nlayers, taps, repeat)
    return _CACHE[key]


def _make_in_maps(inputs):
    qa = np.asarray(inputs["qa_embed_data"])
    qd = np.asarray(inputs["q_embed_data"])
    al = np.asarray(inputs["alphas"])
    ge = np.asarray(inputs["gumbel_E"])
    a0f = al[..., 0]; a1f = al[..., 1]
    e0f = ge[..., 0]; e1f = ge[..., 1]
    i_ = np.arange(S)
    shared = {
        "kwt": np.asarray(inputs["kW"]).transpose(0, 2, 1),
        "vwt": np.asarray(inputs["vW"]).transpose(0, 2, 1),
        "owt": np.asarray(inputs["oW"]).transpose(0, 2, 1),
        "w1t": np.asarray(inputs["w1"]).transpose(0, 2, 1),
        "w2t": np.asarray(inputs["w2"]).transpose(0, 2, 1),
        "a0f": a0f, "a1f": a1f, "e0f": e0f, "e1f": e1f,
        "a0r": a0f[:, :, ::-1], "a1r": a1f[:, :, ::-1],
        "e0r": e0f[:, :, ::-1], "e1r": e1f[:, :, ::-1],
        "gam": np.asarray(inputs["gammas"]).reshape(1, LN_ * H),
        "posn": -np.abs(i_[:, None] - i_[None, :]),
    }
    import ml_dtypes
    casts = {"w1t": ml_dtypes.bfloat16, "w2t": ml_dtypes.bfloat16,
             "owt": ml_dtypes.bfloat16, "posn": np.float16}
    shared = {k: np.ascontiguousarray(v, dtype=casts.get(k, np.float32))
              for k, v in shared.items()}

    def feat_major(x, c):
        pair = np.asarray(x[NB * c:NB * c + NB])        # [2, S, D]
        return np.ascontiguousarray(
            pair.transpose(2, 0, 1).reshape(D, TOK), dtype=np.float32)

    in_maps = []
    for c in range(8):
        m = dict(shared)
        m["xqa"] = feat_major(qa, c)
        m["xq"] = feat_major(qd, c)
        in_maps.append(m)
    return in_maps


def _gather_out(results):
    outs = []
    for r in results:
        o = r["out"].reshape(D, NB, S).transpose(1, 2, 0)
        outs.append(o)
    return np.ascontiguousarray(np.concatenate(outs, axis=0))


def kernel(**inputs):
    from concourse.bass_utils import run_bass_kernel_spmd
    nc, _ = _get_nc()
    in_maps = _make_in_maps(inputs)
    res = run_bass_kernel_spmd(nc, in_maps, core_ids=list(range(8)))
    return _gather_out(res.results)


# revision 45
# speedup vs baseline: 1.3707x; 1.0280x over previous
"""Trainium2 Bass kernel for nn_Architecture_50629074485965 (3-layer AKT-style
transformer, B=16 S=512 D=1024 H=8 DFF=4096).

Sharding: data-parallel over batch — 2 batches per core, 8 cores, no
collectives.  Activations are feature-major [D on partitions, tokens free] so
every matmul chains without activation transposes (weights host-pre-
transposed).  Score path (K, q@k) runs in float32r; the value path (V, att,
probs, FFN) runs bf16.  Layer outputs bounce through DRAM.

All tile pools are persistent: tags rotate across batches and layers instead
of pool release/realloc, so the scheduler can overlap batch b1's projections
and attention with batch b0's FFN (PE-heavy vs ACT/DVE-heavy phases).

Every ACT transcendental is Exp or Ln (sqrt(x) = exp(0.5 ln x)) so a single
activation table set serves the whole kernel (no ~2.7us table swaps).

Attention per (b,h), per 128-row q-tile (q-major [q, k] layout):
  psum  = q @ k^T                         (PE f32r)
  e1    = Exp(psum/sqrt(dk))              (ACT, full width)
  e1c   = causal(e1)                      (GPSIMD affine_select, width w)
  r1    = sum_j e1*dam01                  (DVE stt accum -> throwaway edam;
                                           dam01 = u8 row-window gather from a
                                           per-head Toeplitz vector)
  cum   = cumsum(e1c)                     (DVE tensor_tensor_scan)
  d2    = (cum - rowtot) * (-|i-j|) >= 0  (DVE stt, posn = -|i-j| in f16)
  te    = exp(-exp(0.5 ln d2 + lgam - 0.5 ln r1))   (ACT Ln/Exp/Exp)
  t2u   = max(te,1e-5) * psum             (DVE stt; diag block causal-masked
                                           in place by GPSIMD)
  e2,r2 = Exp(t2u/sqrt(dk)) + row-sum     (ACT accum_out)
  probs = e2 * (1/max(r2,1e-30)) -> bf16  (DVE)
  probsT blocks: PE transpose -> psum -> sbuf
  att   = v-chunks(lhsT) @ probsT -> feature-major  (PE, bf16)
"""
import sys
sys.path.insert(0, "/opt/trn_rl_repo")
import numpy as np

B, S, D, H, DFF, LN_ = 16, 512, 1024, 8, 4096, 3
DK = D // H
NB = 2
TOK = NB * S
P = 128
ND = D // P      # 8
NQ = S // P      # 4
ISD = 1.0 / float(np.sqrt(DK))
WPAD = 2048

_CACHE = {}


def _build(nlayers=3, taps=(), repeat=1):
    import concourse.bass as bass
    import concourse.mybir as mybir
    from concourse import bacc
    from concourse.tile import TileContext

    dt = mybir.dt
    f32, f32r, bf16, f16, u8, i32 = (dt.float32, dt.float32r, dt.bfloat16,
                                     dt.float16, dt.uint8, dt.int32)
    AF = mybir.ActivationFunctionType
    OP = mybir.AluOpType

    nc = bacc.Bacc(None, target_bir_lowering=False)

    # Every transcendental in this kernel is Exp or Ln. The act-table-load
    # pass picks the first act_info set containing each function, which makes
    # Exp/Ln alternation swap tables every few ops (~2.7us per swap on HW).
    # Steer both to the combined natural_log_exp set by hiding them from the
    # single-function sets (dict identity is the functools.cache singleton;
    # set indices — what walrus consumes — are unchanged).
    from concourse.hw_specs import get_activation_tables
    _tabs = get_activation_tables(nc.m.arch)
    for _name, _fns in _tabs.items():
        if _name != "natural_log_exp_and_others":
            _fns.discard(AF.Exp)
            _fns.discard(AF.Ln)

    def par(name, shape, out=False, dtype=None):
        return nc.declare_dram_parameter(name, list(shape), dtype or f32,
                                         isOutput=out)

    xqa_e = par("xqa", [D, TOK], dtype=f32r)
    xq_e = par("xq", [D, TOK], dtype=f32r)
    kwt_e = par("kwt", [LN_, D, D], dtype=f32r)
    vwt_e = par("vwt", [LN_, D, D], dtype=f32r)
    owt_e = par("owt", [LN_, D, D], dtype=bf16)
    w1t_e = par("w1t", [LN_, D, DFF], dtype=bf16)
    w2t_e = par("w2t", [LN_, DFF, D], dtype=bf16)
    a0f_e = par("a0f", [LN_, H, S]); a1f_e = par("a1f", [LN_, H, S])
    e0f_e = par("e0f", [LN_, H, S]); e1f_e = par("e1f", [LN_, H, S])
    a0r_e = par("a0r", [LN_, H, S]); a1r_e = par("a1r", [LN_, H, S])
    e0r_e = par("e0r", [LN_, H, S]); e1r_e = par("e1r", [LN_, H, S])
    gam_e = par("gam", [1, LN_ * H])
    posn_e = par("posn", [S, S], dtype=f16)
    out_e = par("out", [D, TOK], out=True)
    tap_outs = {}

    with TileContext(nc) as tc:
        pg = tc.alloc_tile_pool(name="glob", bufs=1)
        pdram = tc.alloc_tile_pool(name="dram", bufs=1, space="DRAM")
        psQ = tc.alloc_tile_pool(name="psQ", bufs=3, space="PSUM")
        psT = tc.alloc_tile_pool(name="psT", bufs=1, space="PSUM")
        psAv = tc.alloc_tile_pool(name="psAv", bufs=1, space="PSUM")
        pool = tc.alloc_tile_pool(name="main", bufs=2)

        _dmaq = [nc.sync, nc.scalar]
        _dmac = [0]

        def wdma(out, in_):
            eng = _dmaq[_dmac[0] % len(_dmaq)]
            _dmac[0] += 1
            eng.dma_start(out=out, in_=in_)

        def mm_group(psum_ap, pairs):
            n = len(pairs)
            for i, (lt, rh) in enumerate(pairs):
                nc.tensor.matmul(psum_ap, lt, rh,
                                 start=(i == 0), stop=(i == n - 1))

        # ---------------- constants (global pool) ----------------
        ident_f = pg.tile([P, P], f32, name="t", tag="identf")
        nc.gpsimd.memset(ident_f[:], 0.0)
        nc.gpsimd.affine_select(
            out=ident_f[:], in_=ident_f[:], compare_op=OP.not_equal,
            fill=1.0, base=0, channel_multiplier=1, pattern=[[-1, P]])
        ident_bf = pg.tile([P, P], bf16, name="t", tag="identbf")
        nc.vector.tensor_copy(ident_bf[:], ident_f[:])

        ones_f = pg.tile([P, 1], f32, name="t", tag="onesf")
        nc.gpsimd.memset(ones_f[:], 1.0)
        ones_col = pg.tile([P, 1], f32r, name="t", tag="ones")
        nc.vector.tensor_copy(ones_col[:], ones_f[:])
        eps5 = pg.tile([P, 1], f32, name="t", tag="eps5")
        nc.gpsimd.memset(eps5[:], 1e-5)

        posn = []
        for qt in range(NQ):
            t = pg.tile([P, S], f16, name="t", tag=f"posn{qt}")
            nc.sync.dma_start(out=t[:], in_=posn_e[qt * P:(qt + 1) * P, :])
            posn.append(t)

        idxt = []
        for h in range(H):
            t = pg.tile([P, 1], i32, name="t", tag=f"idx{h}")
            nc.gpsimd.iota(t[:], pattern=[[1, 1]],
                           base=h * WPAD + (S - 1) - P * (NQ - 1),
                           channel_multiplier=-1)
            idxt.append(t)

        grow = pg.tile([1, LN_ * H], f32, name="t", tag="grow")
        nc.sync.dma_start(out=grow[:], in_=gam_e[:])
        one_c = pg.tile([P, 1], f32, name="t", tag="one_c")
        nc.gpsimd.memset(one_c[:], 1.0)
        # softplus(x) = ln(1 + exp(x)) computed manually (no Softplus table)
        gsp = pg.tile([1, LN_ * H], f32, name="t", tag="gsp")
        nc.scalar.activation(gsp[:], grow[:], AF.Exp)
        nc.scalar.activation(gsp[:], gsp[:], AF.Ln, bias=one_c[:1, :])
        # lgam = ln(softplus(gamma)); te = exp(-exp(0.5*ln(d2)-0.5*ln(r1)+lgam))
        lgam = pg.tile([1, LN_ * H], f32, name="t", tag="lgam")
        nc.scalar.activation(lgam[:], gsp[:], AF.Ln)
        lgam_bc = []
        for i in range(LN_ * H):
            t = pg.tile([P, 1], f32, name="t", tag=f"gbc{i}")
            nc.gpsimd.partition_broadcast(t[:], lgam[0:1, i:i + 1])
            lgam_bc.append(t)

        y_dram = pdram.tile([D, TOK], f32r, name="t", tag="ydram")
        x1_dram = pdram.tile([D, TOK], f32r, name="t", tag="x1dram")

        # ---------------- helpers ----------------
        def dam_prep(l):
            wdam = pdram.tile([1, H * WPAD], u8, name="t", tag="wdam",
                              bufs=2)

            def half(a0e, a1e, e0e, e1e):
                tA = pool.tile([H, S], f32, name="t", tag="tmpA", bufs=4)
                tB = pool.tile([H, S], f32, name="t", tag="tmpB", bufs=3)
                tC = pool.tile([H, S], f32, name="t", tag="tmpA", bufs=4)
                tD = pool.tile([H, S], f32, name="t", tag="tmpB", bufs=3)
                nc.sync.dma_start(out=tA[:], in_=e0e[l])
                nc.sync.dma_start(out=tB[:], in_=e1e[l])
                nc.scalar.activation(tA[:], tA[:], AF.Ln, bias=eps5[:H, :])
                nc.scalar.activation(tB[:], tB[:], AF.Ln, bias=eps5[:H, :])
                nc.vector.tensor_tensor(tA[:], tA[:], tB[:], OP.subtract)
                nc.sync.dma_start(out=tC[:], in_=a1e[l])
                nc.sync.dma_start(out=tD[:], in_=a0e[l])
                nc.vector.tensor_tensor(tC[:], tC[:], tD[:], OP.subtract)
                nc.vector.tensor_tensor(tA[:], tA[:], tC[:], OP.add)
                c = pool.tile([H, S], u8, name="t", tag="edam", bufs=2)
                nc.vector.tensor_scalar(c[:], tA[:], 0.0, None, OP.is_gt)
                return c

            cf = half(a0f_e, a1f_e, e0f_e, e1f_e)
            cr = half(a0r_e, a1r_e, e0r_e, e1r_e)
            dst_r = bass.AP(tensor=wdam.tensor, offset=0,
                            ap=[[WPAD, H], [1, S - 1]])
            dst_f = bass.AP(tensor=wdam.tensor, offset=S - 1,
                            ap=[[WPAD, H], [1, S]])
            nc.sync.dma_start(out=dst_r, in_=cr[:, 0:S - 1])
            nc.sync.dma_start(out=dst_f, in_=cf[:])
            return wdam

        def layernorm(r_t, dsts):
            """r_t: 8 [P,S] f32r tiles; writes (x-mu)/sigma into dsts APs."""
            s1 = psT.tile([1, S], f32, name="t", tag="pt0")
            mm_group(s1[:], [(ones_col[:], r_t[od][:]) for od in range(ND)])
            s2 = psT.tile([1, S], f32, name="t", tag="pt1")
            for od in range(ND):
                sq = pool.tile([P, S], f32r, name="t", tag="tmpA", bufs=4)
                nc.vector.tensor_tensor(sq[:], r_t[od][:], r_t[od][:],
                                        OP.mult)
                nc.tensor.matmul(s2[:], ones_col[:], sq[:],
                                 start=(od == 0), stop=(od == ND - 1))
            mean = pool.tile([1, S], f32, name="t", tag="lnr0", bufs=2)
            nc.vector.tensor_scalar(mean[:], s1[:], 1.0 / D, None, OP.mult)
            msq = pool.tile([1, S], f32, name="t", tag="lnr1", bufs=2)
            nc.vector.tensor_scalar(msq[:], s2[:], 1.0 / D, None, OP.mult)
            m2 = pool.tile([1, S], f32, name="t", tag="lnr2", bufs=2)
            nc.vector.tensor_tensor(m2[:], mean[:], mean[:], OP.mult)
            nc.vector.tensor_tensor(msq[:], msq[:], m2[:], OP.subtract)
            # rstd = exp(-0.5*ln(var+eps)) — stays in the ln/exp table set
            nc.scalar.activation(msq[:], msq[:], AF.Ln, bias=eps5[:1, :])
            nc.scalar.activation(m2[:], msq[:], AF.Exp, scale=-0.5)
            nc.vector.tensor_scalar(mean[:], mean[:], -1.0, None, OP.mult)
            nc.vector.tensor_tensor(mean[:], mean[:], m2[:], OP.mult)
            Ab = pool.tile([P, S], f32, name="t", tag="Ab", bufs=1)
            nc.gpsimd.partition_broadcast(Ab[:], m2[:])
            Cb = pool.tile([P, S], f32, name="t", tag="Cb", bufs=1)
            nc.gpsimd.partition_broadcast(Cb[:], mean[:])
            for od in range(ND):
                t1 = pool.tile([P, S], f32, name="t", tag="lnt", bufs=1)
                nc.vector.tensor_tensor(t1[:], r_t[od][:], Ab[:], OP.mult)
                nc.gpsimd.tensor_tensor(dsts[od], t1[:], Cb[:], OP.add)

        def attention_head(l, bmask, h, K, V, att_dst, damG):
            pst = [psT.tile([P, S], bf16, name="t", tag=f"pt{kc}")
                   for kc in range(NQ)]
            ktile = K[h]
            for qt in range(NQ):
                w = P * (qt + 1)
                ps = psQ.tile([P, S], f32, name="t", tag="qk")
                mm_group(ps[:], [(ktile[:, qt * P:qt * P + P], ktile[:])])
                doff = P * (NQ - 1) - P * qt
                e1 = pool.tile([P, S], bf16, name="t", tag="e1", bufs=2)
                nc.scalar.activation(e1[:], ps[:], AF.Exp, scale=ISD)
                e1c = pool.tile([P, S], bf16, name="t", tag="e1c", bufs=2)
                nc.gpsimd.affine_select(
                    out=e1c[:, :w], in_=e1[:, :w], compare_op=OP.is_gt,
                    fill=0.0, base=qt * P + bmask, channel_multiplier=1,
                    pattern=[[-1, w]])
                r1 = pool.tile([P, 1], f32, name="t", tag="sm_r1")
                edam = pool.tile([P, S], bf16, name="t", tag="edam", bufs=2)
                nc.vector.scalar_tensor_tensor(
                    edam[:], e1[:], 1.0, damG[:, doff:doff + S],
                    OP.mult, OP.mult, accum_out=r1[:])
                cum = pool.tile([P, S], f32, name="t", tag="tmpB", bufs=3)
                nc.vector.tensor_tensor_scan(
                    cum[:, :w], e1c[:, :w], e1c[:, :w], 0.0, OP.add, OP.bypass)
                lnr1 = pool.tile([P, 1], f32, name="t", tag="sm_rc1")
                nc.scalar.activation(lnr1[:], r1[:], AF.Ln)
                brow = pool.tile([P, 1], f32, name="t", tag="sm_brow")
                nc.vector.scalar_tensor_tensor(
                    brow[:], lnr1[:], -0.5, lgam_bc[l * H + h][:],
                    OP.mult, OP.add)
                d2 = pool.tile([P, S], f32, name="t", tag="tmpA", bufs=4)
                nc.vector.scalar_tensor_tensor(
                    d2[:, :w], cum[:, :w], cum[:, w - 1:w], posn[qt][:, :w],
                    OP.subtract, OP.mult)
                dist = pool.tile([P, S], f32, name="t", tag="tmpB", bufs=3)
                nc.scalar.activation(dist[:, :w], d2[:, :w], AF.Ln)
                sga = pool.tile([P, S], f32, name="t", tag="tmpA", bufs=4)
                nc.scalar.activation(sga[:, :w], dist[:, :w], AF.Exp,
                                     scale=0.5, bias=brow[:])
                te = pool.tile([P, S], f32, name="t", tag="tmpB", bufs=3)
                nc.scalar.activation(te[:, :w], sga[:, :w], AF.Exp,
                                     scale=-1.0)
                t2u = pool.tile([P, S], f32, name="t", tag="tmpA", bufs=4)
                nc.vector.scalar_tensor_tensor(
                    t2u[:, :w], te[:, :w], 1e-5, ps[:, :w], OP.max, OP.mult)
                # causal boundary only cuts the 128-wide diagonal block;
                # mask it in place instead of re-writing the full width.
                nc.gpsimd.affine_select(
                    out=t2u[:, w - P:w], in_=t2u[:, w - P:w],
                    compare_op=OP.is_gt, fill=-1e30, base=bmask,
                    channel_multiplier=1, pattern=[[-1, P]])
                e2 = pool.tile([P, S], bf16, name="t", tag="tmpB", bufs=3)
                r2 = pool.tile([P, 1], f32, name="t", tag="sm_r2")
                nc.scalar.activation(e2[:, :w], t2u[:, :w], AF.Exp,
                                     scale=ISD, accum_out=r2[:])
                nc.vector.tensor_scalar(r2[:], r2[:], 1e-30, None, OP.max)
                rec2 = pool.tile([P, 1], f32, name="t", tag="sm_rc2")
                nc.vector.reciprocal(rec2[:], r2[:])
                pr = pool.tile([P, S], bf16, name="t", tag="probs", bufs=2)
                nc.vector.tensor_scalar(pr[:, :w], e2[:, :w], rec2[:],
                                        None, OP.mult)
                for kc in range(qt + 1):
                    nc.tensor.transpose(
                        pst[kc][:, qt * P:qt * P + P],
                        pr[:, kc * P:kc * P + P], ident_bf[:])
            prT = []
            for kc in range(NQ):
                t = pool.tile([P, S], bf16, name="t", tag=f"prT{kc}", bufs=1)
                nc.vector.tensor_copy(t[:, kc * P:], pst[kc][:, kc * P:])
                prT.append(t)
            pav = psAv.tile([P, S], f32, name="t", tag="av")
            for kc in range(NQ):
                nc.tensor.matmul(
                    pav[:, kc * P:], V[kc][:, h * DK:(h + 1) * DK],
                    prT[kc][:, kc * P:],
                    start=(kc == 0), stop=(kc == NQ - 1))
            nc.scalar.copy(att_dst, pav[:])

        def layer(l, bmask, apply_pos, xsrc_dram, vals_src, out_dram,
                  final=False):
            """xsrc_dram: [D, TOK] DRAM source for the query/key input.
            vals_src: 'self' or a DRAM tile to stream per b.
            out_dram: DRAM target AP base for the layer output."""
            wdam = dam_prep(l)
            damGs = []
            for h in range(H):
                g = pool.tile([P, 2 * S - 1 - P], u8, name="t", tag=f"damG{h}",
                              bufs=1)
                nc.gpsimd.indirect_dma_start(
                    out=g[:], out_offset=None, in_=wdam[:],
                    in_offset=bass.IndirectOffsetOnAxis(
                        ap=idxt[h][:, :1], axis=1))
                damGs.append(g)
            for b in range(NB):
                bs = b * S
                xq_tiles = []
                for idt in range(ND):
                    t = pool.tile([P, S], f32r, name="t", tag=f"xa{idt}",
                                  bufs=1)
                    wdma(t[:],
                         xsrc_dram[idt * P:(idt + 1) * P, bs:bs + S])
                    xq_tiles.append(t)
                # ---- K projection (q==k), kwt streamed in od-halves
                K = []
                for half in range(2):
                    wk = []
                    for idt in range(ND):
                        t = pool.tile([P, S], f32r, name="t",
                                      tag=f"kw{idt}", bufs=1)
                        wdma(
                            t[:],
                            kwt_e[l, idt * P:(idt + 1) * P,
                                      half * S:(half + 1) * S])
                        wk.append(t)
                    for oc in range(4):
                        od = half * 4 + oc
                        ps = psQ.tile([P, S], f32, name="t", tag="qk")
                        mm_group(ps[:], [
                            (wk[idt][:, oc * P:(oc + 1) * P],
                             xq_tiles[idt][:]) for idt in range(ND)])
                        kt = pool.tile([P, S], f32r, name="t", tag=f"K{od}",
                                       bufs=1)
                        nc.scalar.copy(kt[:], ps[:])
                        K.append(kt)
                # ---- VALS for v-projection
                if vals_src == "self":
                    vals = [xq_tiles[idt][:] for idt in range(ND)]
                else:
                    vt = []
                    for idt in range(ND):
                        t = pool.tile([P, S], f32r, name="t", tag=f"r{idt}",
                                      bufs=1)
                        wdma(
                            t[:],
                            vals_src[idt * P:(idt + 1) * P, bs:bs + S])
                        vt.append(t)
                    vals = [t[:] for t in vt]
                # ---- V projection (token-major), vwt streamed in d-halves
                V = [pool.tile([P, D], bf16, name="t", tag=f"V{st}", bufs=1)
                     for st in range(NQ)]
                for half in range(2):
                    wv = []
                    for idt in range(ND):
                        t = pool.tile([P, S], f32r, name="t",
                                      tag=f"kw{idt}", bufs=1)
                        wdma(
                            t[:],
                            vwt_e[l, idt * P:(idt + 1) * P,
                                      half * S:(half + 1) * S])
                        wv.append(t)
                    for st in range(NQ):
                        ps = psQ.tile([P, S], f32, name="t", tag="qk")
                        mm_group(ps[:], [
                            (vals[idt][:, st * P:(st + 1) * P], wv[idt][:])
                            for idt in range(ND)])
                        nc.scalar.copy(
                            V[st][:, half * S:(half + 1) * S], ps[:])
                # ---- attention heads
                att = [pool.tile([P, S], bf16, name="t", tag=f"att{od}",
                                 bufs=4)
                       for od in range(ND)]
                for h in range(H):
                    attention_head(l, bmask, h, K, V, att[h][:], damGs[h])
                # ---- o-projection + residual, owt streamed in od-halves
                r_t = []
                for half in range(2):
                    wo = []
                    for idt in range(ND):
                        t = pool.tile([P, S], bf16, name="t",
                                      tag=f"wbig{idt}", bufs=2)
                        wdma(
                            t[:],
                            owt_e[l, idt * P:(idt + 1) * P,
                                      half * S:(half + 1) * S])
                        wo.append(t)
                    for oc in range(4):
                        od = half * 4 + oc
                        ps = psQ.tile([P, S], f32, name="t", tag="qk")
                        mm_group(ps[:], [
                            (wo[idt][:, oc * P:(oc + 1) * P], att[idt][:])
                            for idt in range(ND)])
                        rt = pool.tile([P, S], f32r, name="t",
                                       tag=f"r{od}", bufs=1)
                        nc.vector.tensor_tensor(
                            rt[:], xq_tiles[od][:], ps[:], OP.add)
                        r_t.append(rt)
                # ---- LN1
                if apply_pos:
                    xp = [pg.tile([P, S], f32r, name="t", tag=f"xp{od}")
                          for od in range(ND)]
                    layernorm(r_t, [t[:] for t in xp])
                else:
                    ot = [pool.tile([P, S], f32 if final else f32r, name="t",
                                    tag="outt", bufs=1)
                          for _ in range(ND)]
                    layernorm(r_t, [t[:] for t in ot])
                    for od in range(ND):
                        nc.sync.dma_start(
                            out=out_dram[od * P:(od + 1) * P, bs:bs + S],
                            in_=ot[od][:])
                    continue

                # ---- FFN + LN2 (bf16 weights and activations)
                xpb = []
                for od in range(ND):
                    t = pool.tile([P, S], bf16, name="t", tag=f"xpb{od}",
                                  bufs=1)
                    nc.vector.tensor_copy(t[:], xp[od][:])
                    xpb.append(t)
                h1 = []
                for fc2 in range(4):
                    w1c = []
                    for idt in range(ND):
                        t = pool.tile([P, 2 * S], bf16, name="t",
                                      tag=f"wbig{idt}", bufs=2)
                        wdma(
                            t[:],
                            w1t_e[l, idt * P:(idt + 1) * P,
                                      fc2 * 2 * S:(fc2 + 1) * 2 * S])
                        w1c.append(t)
                    for fl in range(8):
                        ft = fc2 * 8 + fl
                        ps = psQ.tile([P, S], f32, name="t", tag="qk")
                        mm_group(ps[:], [
                            (w1c[idt][:, fl * P:(fl + 1) * P], xpb[idt][:])
                            for idt in range(ND)])
                        ht = pool.tile([P, S], bf16, name="t",
                                       tag=f"att{ft % 8}", bufs=4)
                        nc.scalar.activation(ht[:], ps[:], AF.Relu)
                        h1.append(ht)
                r_t = []
                for og in range(2):
                    pso = [psT.tile([P, S], f32, name="t", tag=f"pt{oc}")
                           for oc in range(4)]
                    for fc in range(8):
                        w2c = []
                        for fl in range(4):
                            ft = fc * 4 + fl
                            t = pool.tile([P, S], bf16, name="t",
                                          tag=f"wbig{4 + fl}", bufs=2)
                            wdma(
                                t[:],
                                w2t_e[l, ft * P:(ft + 1) * P,
                                          og * S:(og + 1) * S])
                            w2c.append(t)
                        for fl in range(4):
                            ft = fc * 4 + fl
                            for oc in range(4):
                                nc.tensor.matmul(
                                    pso[oc][:],
                                    w2c[fl][:, oc * P:(oc + 1) * P],
                                    h1[ft][:],
                                    start=(fc == 0 and fl == 0),
                                    stop=(fc == 7 and fl == 3))
                    for oc in range(4):
                        od = og * 4 + oc
                        rt = pool.tile([P, S], f32r, name="t", tag=f"r{od}",
                                       bufs=1)
                        nc.vector.tensor_tensor(
                            rt[:], xp[od][:], pso[oc][:], OP.add)
                        r_t.append(rt)
                ot = [pool.tile([P, S], f32 if final else f32r, name="t",
                                tag="outt", bufs=1)
                      for _ in range(ND)]
                layernorm(r_t, [t[:] for t in ot])
                for od in range(ND):
                    nc.sync.dma_start(
                        out=out_dram[od * P:(od + 1) * P, bs:bs + S],
                        in_=ot[od][:])

        # ================= driver =================
        for _rep in range(repeat):
            layer(0, 1, True, xqa_e, "self", y_dram)
            if nlayers >= 2:
                layer(1, 1, False, xq_e, "self", x1_dram)
            if nlayers >= 3:
                layer(2, 0, True, x1_dram, y_dram, out_e, final=True)
            if nlayers == 1:
                nc.gpsimd.dma_start(out=out_e[:], in_=y_dram[:])
            elif nlayers == 2:
                nc.gpsimd.dma_start(out=out_e[:], in_=x1_dram[:])

        pool.release()
        psAv.release()
        psT.release()
        psQ.release()
        pdram.release()
        pg.release()

    nc.finalize()
    return nc, tap_outs


def _get_nc(nlayers=3, taps=(), repeat=1):
    key = (nlayers, tuple(sorted(taps)), repeat)
    if key not in _CACHE:
        _CACHE[key] = _build(nlayers, taps, repeat)
    return _CACHE[key]


def _make_in_maps(inputs):
    qa = np.asarray(inputs["qa_embed_data"])
    qd = np.asarray(inputs["q_embed_data"])
    al = np.asarray(inputs["alphas"])
    ge = np.asarray(inputs["gumbel_E"])
    a0f = al[..., 0]; a1f = al[..., 1]
    e0f = ge[..., 0]; e1f = ge[..., 1]
    i_ = np.arange(S)
    shared = {
        "kwt": np.asarray(inputs["kW"]).transpose(0, 2, 1),
        "vwt": np.asarray(inputs["vW"]).transpose(0, 2, 1),
        "owt": np.asarray(inputs["oW"]).transpose(0, 2, 1),
        "w1t": np.asarray(inputs["w1"]).transpose(0, 2, 1),
        "w2t": np.asarray(inputs["w2"]).transpose(0, 2, 1),
        "a0f": a0f, "a1f": a1f, "e0f": e0f, "e1f": e1f,
        "a0r": a0f[:, :, ::-1], "a1r": a1f[:, :, ::-1],
        "e0r": e0f[:, :, ::-1], "e1r": e1f[:, :, ::-1],
        "gam": np.asarray(inputs["gammas"]).reshape(1, LN_ * H),
        "posn": -np.abs(i_[:, None] - i_[None, :]),
    }
    import ml_dtypes
    casts = {"w1t": ml_dtypes.bfloat16, "w2t": ml_dtypes.bfloat16,
             "owt": ml_dtypes.bfloat16, "posn": np.float16}
    shared = {k: np.ascontiguousarray(v, dtype=casts.get(k, np.float32))
              for k, v in shared.items()}

    def feat_major(x, c):
        pair = np.asarray(x[NB * c:NB * c + NB])        # [2, S, D]
        return np.ascontiguousarray(
            pair.transpose(2, 0, 1).reshape(D, TOK), dtype=np.float32)

    in_maps = []
    for c in range(8):
        m = dict(shared)
        m["xqa"] = feat_major(qa, c)
        m["xq"] = feat_major(qd, c)
        in_maps.append(m)
    return in_maps


def _gather_out(results):
    outs = []
    for r in results:
        o = r["out"].reshape(D, NB, S).transpose(1, 2, 0)
        outs.append(o)
    return np.ascontiguousarray(np.concatenate(outs, axis=0))


def kernel(**inputs):
    from concourse.bass_utils import run_bass_kernel_spmd
    nc, _ = _get_nc()
    in_maps = _make_in_maps(inputs)
    res = run_bass_kernel_spmd(nc, in_maps, core_ids=list(range(8)))
    return _gather_out(res.results)


# revision 55
# speedup vs baseline: 1.3720x; 1.0010x over previous
"""Trainium2 Bass kernel for nn_Architecture_50629074485965 (3-layer AKT-style
transformer, B=16 S=512 D=1024 H=8 DFF=4096).

Sharding: data-parallel over batch — 2 batches per core, 8 cores, no
collectives.  Activations are feature-major [D on partitions, tokens free] so
every matmul chains without activation transposes (weights host-pre-
transposed).  Score path (K, q@k) runs in float32r; the value path (V, att,
probs, FFN) runs bf16.  Layer outputs bounce through DRAM.

All tile pools are persistent: tags rotate across batches and layers instead
of pool release/realloc, so the scheduler can overlap batch b1's projections
and attention with batch b0's FFN (PE-heavy vs ACT/DVE-heavy phases).

Every ACT transcendental is Exp or Ln (sqrt(x) = exp(0.5 ln x)) so a single
activation table set serves the whole kernel (no ~2.7us table swaps).

Attention per (b,h), per 128-row q-tile (q-major [q, k] layout):
  psum  = q @ k^T                         (PE f32r)
  e1    = Exp(psum/sqrt(dk))              (ACT, full width)
  e1c   = causal(e1)                      (GPSIMD affine_select, width w)
  r1    = sum_j e1*dam01                  (DVE stt accum -> throwaway edam;
                                           dam01 = u8 row-window gather from a
                                           per-head Toeplitz vector)
  cum   = cumsum(e1c)                     (DVE tensor_tensor_scan)
  d2    = (cum - rowtot) * (-|i-j|) >= 0  (DVE stt, posn = -|i-j| in f16)
  te    = exp(-exp(0.5 ln d2 + lgam - 0.5 ln r1))   (ACT Ln/Exp/Exp)
  t2u   = max(te,1e-5) * psum             (DVE stt; diag block causal-masked
                                           in place by GPSIMD)
  e2,r2 = Exp(t2u/sqrt(dk)) + row-sum     (ACT accum_out)
  probs = e2 * (1/max(r2,1e-30)) -> bf16  (DVE)
  probsT blocks: PE transpose -> psum -> sbuf
  att   = v-chunks(lhsT) @ probsT -> feature-major  (PE, bf16)
"""
import sys
sys.path.insert(0, "/opt/trn_rl_repo")
import numpy as np

B, S, D, H, DFF, LN_ = 16, 512, 1024, 8, 4096, 3
DK = D // H
NB = 2
TOK = NB * S
P = 128
ND = D // P      # 8
NQ = S // P      # 4
ISD = 1.0 / float(np.sqrt(DK))
WPAD = 2048

_CACHE = {}


def _build(nlayers=3, taps=(), repeat=1):
    import concourse.bass as bass
    import concourse.mybir as mybir
    from concourse import bacc
    from concourse.tile import TileContext

    dt = mybir.dt
    f32, f32r, bf16, f16, u8, i32 = (dt.float32, dt.float32r, dt.bfloat16,
                                     dt.float16, dt.uint8, dt.int32)
    AF = mybir.ActivationFunctionType
    OP = mybir.AluOpType

    nc = bacc.Bacc(None, target_bir_lowering=False)

    # Every transcendental in this kernel is Exp or Ln. The act-table-load
    # pass picks the first act_info set containing each function, which makes
    # Exp/Ln alternation swap tables every few ops (~2.7us per swap on HW).
    # Steer both to the combined natural_log_exp set by hiding them from the
    # single-function sets (dict identity is the functools.cache singleton;
    # set indices — what walrus consumes — are unchanged).
    from concourse.hw_specs import get_activation_tables
    _tabs = get_activation_tables(nc.m.arch)
    for _name, _fns in _tabs.items():
        if _name != "natural_log_exp_and_others":
            _fns.discard(AF.Exp)
            _fns.discard(AF.Ln)

    def par(name, shape, out=False, dtype=None):
        return nc.declare_dram_parameter(name, list(shape), dtype or f32,
                                         isOutput=out)

    xqa_e = par("xqa", [D, TOK], dtype=f32r)
    xq_e = par("xq", [D, TOK], dtype=f32r)
    kwt_e = par("kwt", [LN_, D, D], dtype=f32r)
    vwt_e = par("vwt", [LN_, D, D], dtype=f32r)
    owt_e = par("owt", [LN_, D, D], dtype=bf16)
    w1t_e = par("w1t", [LN_, D, DFF], dtype=bf16)
    w2t_e = par("w2t", [LN_, DFF, D], dtype=bf16)
    a0f_e = par("a0f", [LN_, H, S]); a1f_e = par("a1f", [LN_, H, S])
    e0f_e = par("e0f", [LN_, H, S]); e1f_e = par("e1f", [LN_, H, S])
    a0r_e = par("a0r", [LN_, H, S]); a1r_e = par("a1r", [LN_, H, S])
    e0r_e = par("e0r", [LN_, H, S]); e1r_e = par("e1r", [LN_, H, S])
    gam_e = par("gam", [1, LN_ * H])
    posn_e = par("posn", [S, S], dtype=f16)
    out_e = par("out", [D, TOK], out=True)
    tap_outs = {}

    with TileContext(nc) as tc:
        pg = tc.alloc_tile_pool(name="glob", bufs=1)
        pdram = tc.alloc_tile_pool(name="dram", bufs=1, space="DRAM")
        psQ = tc.alloc_tile_pool(name="psQ", bufs=3, space="PSUM")
        psT = tc.alloc_tile_pool(name="psT", bufs=1, space="PSUM")
        psAv = tc.alloc_tile_pool(name="psAv", bufs=1, space="PSUM")
        pool = tc.alloc_tile_pool(name="main", bufs=2)

        _dmaq = [nc.sync, nc.scalar]
        _dmac = [0]

        def wdma(out, in_):
            eng = _dmaq[_dmac[0] % len(_dmaq)]
            _dmac[0] += 1
            eng.dma_start(out=out, in_=in_)

        def mm_group(psum_ap, pairs):
            n = len(pairs)
            for i, (lt, rh) in enumerate(pairs):
                nc.tensor.matmul(psum_ap, lt, rh,
                                 start=(i == 0), stop=(i == n - 1))

        # ---------------- constants (global pool) ----------------
        ident_f = pg.tile([P, P], f32, name="t", tag="identf")
        nc.gpsimd.memset(ident_f[:], 0.0)
        nc.gpsimd.affine_select(
            out=ident_f[:], in_=ident_f[:], compare_op=OP.not_equal,
            fill=1.0, base=0, channel_multiplier=1, pattern=[[-1, P]])
        ident_bf = pg.tile([P, P], bf16, name="t", tag="identbf")
        nc.vector.tensor_copy(ident_bf[:], ident_f[:])

        ones_f = pg.tile([P, 1], f32, name="t", tag="onesf")
        nc.gpsimd.memset(ones_f[:], 1.0)
        ones_col = pg.tile([P, 1], f32r, name="t", tag="ones")
        nc.vector.tensor_copy(ones_col[:], ones_f[:])
        eps5 = pg.tile([P, 1], f32, name="t", tag="eps5")
        nc.gpsimd.memset(eps5[:], 1e-5)

        posn = []
        for qt in range(NQ):
            t = pg.tile([P, S], f16, name="t", tag=f"posn{qt}")
            nc.sync.dma_start(out=t[:], in_=posn_e[qt * P:(qt + 1) * P, :])
            posn.append(t)

        idxt = []
        for h in range(H):
            t = pg.tile([P, 1], i32, name="t", tag=f"idx{h}")
            nc.gpsimd.iota(t[:], pattern=[[1, 1]],
                           base=h * WPAD + (S - 1) - P * (NQ - 1),
                           channel_multiplier=-1)
            idxt.append(t)

        grow = pg.tile([1, LN_ * H], f32, name="t", tag="grow")
        nc.sync.dma_start(out=grow[:], in_=gam_e[:])
        one_c = pg.tile([P, 1], f32, name="t", tag="one_c")
        nc.gpsimd.memset(one_c[:], 1.0)
        # softplus(x) = ln(1 + exp(x)) computed manually (no Softplus table)
        gsp = pg.tile([1, LN_ * H], f32, name="t", tag="gsp")
        nc.scalar.activation(gsp[:], grow[:], AF.Exp)
        nc.scalar.activation(gsp[:], gsp[:], AF.Ln, bias=one_c[:1, :])
        # lgam = ln(softplus(gamma)); te = exp(-exp(0.5*ln(d2)-0.5*ln(r1)+lgam))
        lgam = pg.tile([1, LN_ * H], f32, name="t", tag="lgam")
        nc.scalar.activation(lgam[:], gsp[:], AF.Ln)
        lgam_bc = []
        for i in range(LN_ * H):
            t = pg.tile([P, 1], f32, name="t", tag=f"gbc{i}")
            nc.gpsimd.partition_broadcast(t[:], lgam[0:1, i:i + 1])
            lgam_bc.append(t)

        y_dram = pdram.tile([D, TOK], f32r, name="t", tag="ydram")
        x1_dram = pdram.tile([D, TOK], f32r, name="t", tag="x1dram")

        # ---------------- helpers ----------------
        def dam_prep(l):
            wdam = pdram.tile([1, H * WPAD], u8, name="t", tag="wdam",
                              bufs=2)

            def half(a0e, a1e, e0e, e1e):
                tA = pool.tile([H, S], f32, name="t", tag="tmpA", bufs=4)
                tB = pool.tile([H, S], f32, name="t", tag="tmpB", bufs=4)
                tC = pool.tile([H, S], f32, name="t", tag="tmpA", bufs=4)
                tD = pool.tile([H, S], f32, name="t", tag="tmpB", bufs=4)
                nc.sync.dma_start(out=tA[:], in_=e0e[l])
                nc.sync.dma_start(out=tB[:], in_=e1e[l])
                nc.scalar.activation(tA[:], tA[:], AF.Ln, bias=eps5[:H, :])
                nc.scalar.activation(tB[:], tB[:], AF.Ln, bias=eps5[:H, :])
                nc.vector.tensor_tensor(tA[:], tA[:], tB[:], OP.subtract)
                nc.sync.dma_start(out=tC[:], in_=a1e[l])
                nc.sync.dma_start(out=tD[:], in_=a0e[l])
                nc.vector.tensor_tensor(tC[:], tC[:], tD[:], OP.subtract)
                nc.vector.tensor_tensor(tA[:], tA[:], tC[:], OP.add)
                c = pool.tile([H, S], u8, name="t", tag="edam", bufs=2)
                nc.vector.tensor_scalar(c[:], tA[:], 0.0, None, OP.is_gt)
                return c

            cf = half(a0f_e, a1f_e, e0f_e, e1f_e)
            cr = half(a0r_e, a1r_e, e0r_e, e1r_e)
            dst_r = bass.AP(tensor=wdam.tensor, offset=0,
                            ap=[[WPAD, H], [1, S - 1]])
            dst_f = bass.AP(tensor=wdam.tensor, offset=S - 1,
                            ap=[[WPAD, H], [1, S]])
            nc.sync.dma_start(out=dst_r, in_=cr[:, 0:S - 1])
            nc.sync.dma_start(out=dst_f, in_=cf[:])
            return wdam

        def layernorm(r_t, dsts):
            """r_t: 8 [P,S] f32r tiles; writes (x-mu)/sigma into dsts APs."""
            s1 = psT.tile([1, S], f32, name="t", tag="pt0")
            mm_group(s1[:], [(ones_col[:], r_t[od][:]) for od in range(ND)])
            s2 = psT.tile([1, S], f32, name="t", tag="pt1")
            for od in range(ND):
                sq = pool.tile([P, S], f32r, name="t", tag="tmpA", bufs=4)
                nc.vector.tensor_tensor(sq[:], r_t[od][:], r_t[od][:],
                                        OP.mult)
                nc.tensor.matmul(s2[:], ones_col[:], sq[:],
                                 start=(od == 0), stop=(od == ND - 1))
            mean = pool.tile([1, S], f32, name="t", tag="lnr0", bufs=2)
            nc.vector.tensor_scalar(mean[:], s1[:], 1.0 / D, None, OP.mult)
            msq = pool.tile([1, S], f32, name="t", tag="lnr1", bufs=2)
            nc.vector.tensor_scalar(msq[:], s2[:], 1.0 / D, None, OP.mult)
            m2 = pool.tile([1, S], f32, name="t", tag="lnr2", bufs=2)
            nc.vector.tensor_tensor(m2[:], mean[:], mean[:], OP.mult)
            nc.vector.tensor_tensor(msq[:], msq[:], m2[:], OP.subtract)
            # rstd = exp(-0.5*ln(var+eps)) — stays in the ln/exp table set
            nc.scalar.activation(msq[:], msq[:], AF.Ln, bias=eps5[:1, :])
            nc.scalar.activation(m2[:], msq[:], AF.Exp, scale=-0.5)
            nc.vector.tensor_scalar(mean[:], mean[:], -1.0, None, OP.mult)
            nc.vector.tensor_tensor(mean[:], mean[:], m2[:], OP.mult)
            Ab = pool.tile([P, S], f32, name="t", tag="Ab", bufs=1)
            nc.gpsimd.partition_broadcast(Ab[:], m2[:])
            Cb = pool.tile([P, S], f32, name="t", tag="Cb", bufs=1)
            nc.gpsimd.partition_broadcast(Cb[:], mean[:])
            for od in range(ND):
                t1 = pool.tile([P, S], f32, name="t", tag="lnt", bufs=1)
                nc.vector.tensor_tensor(t1[:], r_t[od][:], Ab[:], OP.mult)
                nc.gpsimd.tensor_tensor(dsts[od], t1[:], Cb[:], OP.add)

        def attention_head(l, bmask, h, K, V, att_dst, damG):
            pst = [psT.tile([P, S], bf16, name="t", tag=f"pt{kc}")
                   for kc in range(NQ)]
            ktile = K[h]
            for qt in range(NQ):
                w = P * (qt + 1)
                ps = psQ.tile([P, S], f32, name="t", tag="qk")
                mm_group(ps[:], [(ktile[:, qt * P:qt * P + P], ktile[:])])
                doff = P * (NQ - 1) - P * qt
                e1 = pool.tile([P, S], bf16, name="t", tag="e1", bufs=2)
                nc.scalar.activation(e1[:], ps[:], AF.Exp, scale=ISD)
                e1c = pool.tile([P, S], bf16, name="t", tag="e1c", bufs=2)
                nc.gpsimd.affine_select(
                    out=e1c[:, :w], in_=e1[:, :w], compare_op=OP.is_gt,
                    fill=0.0, base=qt * P + bmask, channel_multiplier=1,
                    pattern=[[-1, w]])
                r1 = pool.tile([P, 1], f32, name="t", tag="sm_r1")
                edam = pool.tile([P, S], bf16, name="t", tag="edam", bufs=2)
                nc.vector.scalar_tensor_tensor(
                    edam[:], e1[:], 1.0, damG[:, doff:doff + S],
                    OP.mult, OP.mult, accum_out=r1[:])
                cum = pool.tile([P, S], f32, name="t", tag="tmpB", bufs=4)
                nc.vector.tensor_tensor_scan(
                    cum[:, :w], e1c[:, :w], e1c[:, :w], 0.0, OP.add, OP.bypass)
                lnr1 = pool.tile([P, 1], f32, name="t", tag="sm_rc1")
                nc.scalar.activation(lnr1[:], r1[:], AF.Ln)
                brow = pool.tile([P, 1], f32, name="t", tag="sm_brow")
                nc.vector.scalar_tensor_tensor(
                    brow[:], lnr1[:], -0.5, lgam_bc[l * H + h][:],
                    OP.mult, OP.add)
                d2 = pool.tile([P, S], f32, name="t", tag="tmpA", bufs=4)
                nc.vector.scalar_tensor_tensor(
                    d2[:, :w], cum[:, :w], cum[:, w - 1:w], posn[qt][:, :w],
                    OP.subtract, OP.mult)
                dist = pool.tile([P, S], f32, name="t", tag="tmpB", bufs=4)
                nc.scalar.activation(dist[:, :w], d2[:, :w], AF.Ln)
                sga = pool.tile([P, S], f32, name="t", tag="tmpA", bufs=4)
                nc.scalar.activation(sga[:, :w], dist[:, :w], AF.Exp,
                                     scale=0.5, bias=brow[:])
                te = pool.tile([P, S], f32, name="t", tag="tmpB", bufs=4)
                nc.scalar.activation(te[:, :w], sga[:, :w], AF.Exp,
                                     scale=-1.0)
                t2u = pool.tile([P, S], f32, name="t", tag="tmpA", bufs=4)
                nc.vector.scalar_tensor_tensor(
                    t2u[:, :w], te[:, :w], 1e-5, ps[:, :w], OP.max, OP.mult)
                # causal boundary only cuts the 128-wide diagonal block;
                # mask it in place instead of re-writing the full width.
                nc.gpsimd.affine_select(
                    out=t2u[:, w - P:w], in_=t2u[:, w - P:w],
                    compare_op=OP.is_gt, fill=-1e30, base=bmask,
                    channel_multiplier=1, pattern=[[-1, P]])
                e2 = pool.tile([P, S], bf16, name="t", tag="tmpB", bufs=4)
                r2 = pool.tile([P, 1], f32, name="t", tag="sm_r2")
                nc.scalar.activation(e2[:, :w], t2u[:, :w], AF.Exp,
                                     scale=ISD, accum_out=r2[:])
                nc.vector.tensor_scalar(r2[:], r2[:], 1e-30, None, OP.max)
                rec2 = pool.tile([P, 1], f32, name="t", tag="sm_rc2")
                nc.vector.reciprocal(rec2[:], r2[:])
                pr = pool.tile([P, S], bf16, name="t", tag="probs", bufs=2)
                nc.vector.tensor_scalar(pr[:, :w], e2[:, :w], rec2[:],
                                        None, OP.mult)
                for kc in range(qt + 1):
                    nc.tensor.transpose(
                        pst[kc][:, qt * P:qt * P + P],
                        pr[:, kc * P:kc * P + P], ident_bf[:])
            prT = []
            for kc in range(NQ):
                t = pool.tile([P, S], bf16, name="t", tag=f"prT{kc}", bufs=1)
                nc.vector.tensor_copy(t[:, kc * P:], pst[kc][:, kc * P:])
                prT.append(t)
            pav = psAv.tile([P, S], f32, name="t", tag="av")
            for kc in range(NQ):
                nc.tensor.matmul(
                    pav[:, kc * P:], V[kc][:, h * DK:(h + 1) * DK],
                    prT[kc][:, kc * P:],
                    start=(kc == 0), stop=(kc == NQ - 1))
            nc.scalar.copy(att_dst, pav[:])

        def layer(l, bmask, apply_pos, xsrc_dram, vals_src, out_dram,
                  final=False):
            """xsrc_dram: [D, TOK] DRAM source for the query/key input.
            vals_src: 'self' or a DRAM tile to stream per b.
            out_dram: DRAM target AP base for the layer output."""
            wdam = dam_prep(l)
            damGs = []
            for h in range(H):
                g = pool.tile([P, 2 * S - P], u8, name="t", tag=f"damG{h}",
                              bufs=1)
                nc.gpsimd.indirect_dma_start(
                    out=g[:], out_offset=None, in_=wdam[:],
                    in_offset=bass.IndirectOffsetOnAxis(
                        ap=idxt[h][:, :1], axis=1))
                damGs.append(g)
            for b in range(NB):
                bs = b * S
                xq_tiles = []
                for idt in range(ND):
                    t = pool.tile([P, S], f32r, name="t", tag=f"xa{idt}",
                                  bufs=1)
                    wdma(t[:],
                         xsrc_dram[idt * P:(idt + 1) * P, bs:bs + S])
                    xq_tiles.append(t)
                # ---- K projection (q==k), kwt streamed in od-halves
                K = []
                for half in range(2):
                    wk = []
                    for idt in range(ND):
                        t = pool.tile([P, S], f32r, name="t",
                                      tag=f"kw{idt}", bufs=1)
                        wdma(
                            t[:],
                            kwt_e[l, idt * P:(idt + 1) * P,
                                      half * S:(half + 1) * S])
                        wk.append(t)
                    for oc in range(4):
                        od = half * 4 + oc
                        ps = psQ.tile([P, S], f32, name="t", tag="qk")
                        mm_group(ps[:], [
                            (wk[idt][:, oc * P:(oc + 1) * P],
                             xq_tiles[idt][:]) for idt in range(ND)])
                        kt = pool.tile([P, S], f32r, name="t", tag=f"K{od}",
                                       bufs=1)
                        nc.scalar.copy(kt[:], ps[:])
                        K.append(kt)
                # ---- VALS for v-projection
                if vals_src == "self":
                    vals = [xq_tiles[idt][:] for idt in range(ND)]
                else:
                    vt = []
                    for idt in range(ND):
                        t = pool.tile([P, S], f32r, name="t", tag=f"r{idt}",
                                      bufs=1)
                        wdma(
                            t[:],
                            vals_src[idt * P:(idt + 1) * P, bs:bs + S])
                        vt.append(t)
                    vals = [t[:] for t in vt]
                # ---- V projection (token-major), vwt streamed in d-halves
                V = [pool.tile([P, D], bf16, name="t", tag=f"V{st}", bufs=1)
                     for st in range(NQ)]
                for half in range(2):
                    wv = []
                    for idt in range(ND):
                        t = pool.tile([P, S], f32r, name="t",
                                      tag=f"kw{idt}", bufs=1)
                        wdma(
                            t[:],
                            vwt_e[l, idt * P:(idt + 1) * P,
                                      half * S:(half + 1) * S])
                        wv.append(t)
                    for st in range(NQ):
                        ps = psQ.tile([P, S], f32, name="t", tag="qk")
                        mm_group(ps[:], [
                            (vals[idt][:, st * P:(st + 1) * P], wv[idt][:])
                            for idt in range(ND)])
                        nc.scalar.copy(
                            V[st][:, half * S:(half + 1) * S], ps[:])
                # ---- attention heads
                att = [pool.tile([P, S], bf16, name="t", tag=f"att{od}",
                                 bufs=4)
                       for od in range(ND)]
                for h in range(H):
                    attention_head(l, bmask, h, K, V, att[h][:], damGs[h])
                # ---- o-projection + residual, owt streamed in od-halves
                r_t = []
                for half in range(2):
                    wo = []
                    for idt in range(ND):
                        t = pool.tile([P, S], bf16, name="t",
                                      tag=f"wbig{idt}", bufs=2)
                        wdma(
                            t[:],
                            owt_e[l, idt * P:(idt + 1) * P,
                                      half * S:(half + 1) * S])
                        wo.append(t)
                    for oc in range(4):
                        od = half * 4 + oc
                        ps = psQ.tile([P, S], f32, name="t", tag="qk")
                        mm_group(ps[:], [
                            (wo[idt][:, oc * P:(oc + 1) * P], att[idt][:])
                            for idt in range(ND)])
                        rt = pool.tile([P, S], f32r, name="t",
                                       tag=f"r{od}", bufs=1)
                        nc.vector.tensor_tensor(
                            rt[:], xq_tiles[od][:], ps[:], OP.add)
                        r_t.append(rt)
                # ---- LN1
                if apply_pos:
                    xp = [pg.tile([P, S], f32r, name="t", tag=f"xp{od}")
                          for od in range(ND)]
                    layernorm(r_t, [t[:] for t in xp])
                else:
                    ot = [pool.tile([P, S], f32 if final else f32r, name="t",
                                    tag="outt", bufs=2)
                          for _ in range(ND)]
                    layernorm(r_t, [t[:] for t in ot])
                    for od in range(ND):
                        nc.sync.dma_start(
                            out=out_dram[od * P:(od + 1) * P, bs:bs + S],
                            in_=ot[od][:])
                    continue

                # ---- FFN + LN2 (bf16 weights and activations)
                xpb = []
                for od in range(ND):
                    t = pool.tile([P, S], bf16, name="t", tag=f"xpb{od}",
                                  bufs=1)
                    nc.vector.tensor_copy(t[:], xp[od][:])
                    xpb.append(t)
                h1 = []
                for fc in range(8):
                    w1c = []
                    for idt in range(ND):
                        t = pool.tile([P, S], bf16, name="t",
                                      tag=f"wbig{idt}", bufs=2)
                        wdma(
                            t[:],
                            w1t_e[l, idt * P:(idt + 1) * P,
                                      fc * S:(fc + 1) * S])
                        w1c.append(t)
                    for fl in range(4):
                        ft = fc * 4 + fl
                        ps = psQ.tile([P, S], f32, name="t", tag="qk")
                        mm_group(ps[:], [
                            (w1c[idt][:, fl * P:(fl + 1) * P], xpb[idt][:])
                            for idt in range(ND)])
                        ht = pool.tile([P, S], bf16, name="t",
                                       tag=f"att{ft % 8}", bufs=4)
                        nc.scalar.activation(ht[:], ps[:], AF.Relu)
                        h1.append(ht)
                r_t = []
                for og in range(2):
                    pso = [psT.tile([P, S], f32, name="t", tag=f"pt{oc}")
                           for oc in range(4)]
                    for fc in range(8):
                        w2c = []
                        for fl in range(4):
                            ft = fc * 4 + fl
                            t = pool.tile([P, S], bf16, name="t",
                                          tag=f"wbig{4 + fl}", bufs=2)
                            wdma(
                                t[:],
                                w2t_e[l, ft * P:(ft + 1) * P,
                                          og * S:(og + 1) * S])
                            w2c.append(t)
                        for fl in range(4):
                            ft = fc * 4 + fl
                            for oc in range(4):
                                nc.tensor.matmul(
                                    pso[oc][:],
                                    w2c[fl][:, oc * P:(oc + 1) * P],
                                    h1[ft][:],
                                    start=(fc == 0 and fl == 0),
                                    stop=(fc == 7 and fl == 3))
                    for oc in range(4):
                        od = og * 4 + oc
                        rt = pool.tile([P, S], f32r, name="t", tag=f"r{od}",
                                       bufs=1)
                        nc.vector.tensor_tensor(
                            rt[:], xp[od][:], pso[oc][:], OP.add)
                        r_t.append(rt)
                ot = [pool.tile([P, S], f32 if final else f32r, name="t",
                                tag="outt", bufs=2)
                      for _ in range(ND)]
                layernorm(r_t, [t[:] for t in ot])
                for od in range(ND):
                    nc.sync.dma_start(
                        out=out_dram[od * P:(od + 1) * P, bs:bs + S],
                        in_=ot[od][:])

        # ================= driver =================
        for _rep in range(repeat):
            layer(0, 1, True, xqa_e, "self", y_dram)
            if nlayers >= 2:
                layer(1, 1, False, xq_e, "self", x1_dram)
            if nlayers >= 3:
                layer(2, 0, True, x1_dram, y_dram, out_e, final=True)
            if nlayers == 1:
                nc.gpsimd.dma_start(out=out_e[:], in_=y_dram[:])
            elif nlayers == 2:
                nc.gpsimd.dma_start(out=out_e[:], in_=x1_dram[:])

        pool.release()
        psAv.release()
        psT.release()
        psQ.release()
        pdram.release()
        pg.release()

    nc.finalize()
    return nc, tap_outs


def _get_nc(nlayers=3, taps=(), repeat=1):
    key = (nlayers, tuple(sorted(taps)), repeat)
    if key not in _CACHE:
        _CACHE[key] = _build(nlayers, taps, repeat)
    return _CACHE[key]


def _make_in_maps(inputs):
    qa = np.asarray(inputs["qa_embed_data"])
    qd = np.asarray(inputs["q_embed_data"])
    al = np.asarray(inputs["alphas"])
    ge = np.asarray(inputs["gumbel_E"])
    a0f = al[..., 0]; a1f = al[..., 1]
    e0f = ge[..., 0]; e1f = ge[..., 1]
    i_ = np.arange(S)
    shared = {
        "kwt": np.asarray(inputs["kW"]).transpose(0, 2, 1),
        "vwt": np.asarray(inputs["vW"]).transpose(0, 2, 1),
        "owt": np.asarray(inputs["oW"]).transpose(0, 2, 1),
        "w1t": np.asarray(inputs["w1"]).transpose(0, 2, 1),
        "w2t": np.asarray(inputs["w2"]).transpose(0, 2, 1),
        "a0f": a0f, "a1f": a1f, "e0f": e0f, "e1f": e1f,
        "a0r": a0f[:, :, ::-1], "a1r": a1f[:, :, ::-1],
        "e0r": e0f[:, :, ::-1], "e1r": e1f[:, :, ::-1],
        "gam": np.asarray(inputs["gammas"]).reshape(1, LN_ * H),
        "posn": -np.abs(i_[:, None] - i_[None, :]),
    }
    import ml_dtypes
    casts = {"w1t": ml_dtypes.bfloat16, "w2t": ml_dtypes.bfloat16,
             "owt": ml_dtypes.bfloat16, "posn": np.float16}
    shared = {k: np.ascontiguousarray(v, dtype=casts.get(k, np.float32))
              for k, v in shared.items()}

    def feat_major(x, c):
        pair = np.asarray(x[NB * c:NB * c + NB])        # [2, S, D]
        return np.ascontiguousarray(
            pair.transpose(2, 0, 1).reshape(D, TOK), dtype=np.float32)

    in_maps = []
    for c in range(8):
        m = dict(shared)
        m["xqa"] = feat_major(qa, c)
        m["xq"] = feat_major(qd, c)
        in_maps.append(m)
    return in_maps


def _gather_out(results):
    outs = []
    for r in results:
        o = r["out"].reshape(D, NB, S).transpose(1, 2, 0)
        outs.append(o)
    return np.ascontiguousarray(np.concatenate(outs, axis=0))


def kernel(**inputs):
    from concourse.bass_utils import run_bass_kernel_spmd
    nc, _ = _get_nc()
    in_maps = _make_in_maps(inputs)
    res = run_bass_kernel_spmd(nc, in_maps, core_ids=list(range(8)))
    return _gather_out(res.results)
